# revision 10
# baseline (speedup 1.0000x reference)
"""Two-layer GAT (PyG-style GATConv x2) on 8 Trainium2 NeuronCores.

Sharding: nodes (and their incident edges, by destination) are sharded
across the 8 cores; small weights are replicated. Per-edge gather of
source-node features uses SWDGE dma_gather from a row-major node table in
HBM; the segment-softmax denominator and the weighted message aggregation
are fused into a single dma_scatter_add (SDMA CCE add) per edge block.

Three SPMD launches with host-side concat between them (no collectives):
  1. table0 build:  h0 = x @ W0, alphas -> row table [N, 320]
  2. layer-0 edges: gather/softmax/scatter -> finalize (ELU) -> table1
  3. layer-1 edges: gather/softmax/scatter -> finalize -> output

Softmax max-subtraction is skipped: e-values are O(sigma*5) so exp() is
safely in fp32 range, and the PyG eps (1e-16) is applied identically.
"""

import os

import numpy as np
from contextlib import ExitStack

import concourse.bacc as bacc
import concourse.mybir as mybir
from concourse import tile
from concourse.bass_utils import run_bass_kernel_spmd

fp32 = mybir.dt.float32
i16 = mybir.dt.int16
Alu = mybir.AluOpType
Act = mybir.ActivationFunctionType

NCORES = 8
NEG_SLOPE = 0.2
EPS = 1e-16


def _dims_full():
    return dict(
        N=50000,  # total nodes
        NLOC=6250,  # nodes per core
        NLOC_PAD=6272,  # padded to mult of 128 (>= NLOC+1; row NLOC = trash)
        F_IN=256,
        HID=256,
        H=4,
        DH=64,
        C_OUT=64,
        ELEM0=320,  # table0 row: h(256) | alpha_src(4) | pad(60)
        ELEM1=128,  # table1 row: h1(64) | alpha_src(1) | pad(63)
        SPLIT=32768,  # int16 gather-index split point
        CALL_E=1024,  # edges per gather/scatter call (SWDGE ring caps ~128 desc/engine)
    )


# ---------------------------------------------------------------- launch 1


def build_phase_a(d):
    """Per core: h0 = x_shard @ W0 (+alphas) -> table0 rows + alphaD0 table."""
    nc = bacc.Bacc(None, target_bir_lowering=False, debug=False)
    NP, F, HID, ELEM0 = d["NLOC_PAD"], d["F_IN"], d["HID"], d["ELEM0"]
    assert F == 256 and HID == 256

    xT = nc.dram_tensor("xT", [F, NP], fp32, kind="ExternalInput")
    W0 = nc.dram_tensor("W0", [F, HID], fp32, kind="ExternalInput")
    A0 = nc.dram_tensor("A0", [HID, 8], fp32, kind="ExternalInput")
    eye = nc.dram_tensor("eye", [128, 128], fp32, kind="ExternalInput")
    table0 = nc.dram_tensor("table0", [NP, ELEM0], fp32, kind="ExternalOutput")
    adtab0 = nc.dram_tensor("adtab0", [NP, 64], fp32, kind="ExternalOutput")

    n_t = NP // 512 if NP % 512 == 0 else NP // 512 + 1
    TW = 512

    with tile.TileContext(nc) as tc:
        with (
            tc.tile_pool(name="const", bufs=1) as cpool,
            tc.tile_pool(name="work", bufs=3) as pool,
            tc.tile_pool(name="psum", bufs=1, space="PSUM") as pp,
            tc.tile_pool(name="psum1", bufs=2, space="PSUM") as pp1,
        ):
            w0_sb = [cpool.tile([128, HID], fp32, tag=f"w0_{k}", name=f"w0_{k}") for k in range(2)]
            a0_sb = [cpool.tile([128, 8], fp32, tag=f"a0_{k}", name=f"a0_{k}") for k in range(2)]
            eye_sb = cpool.tile([128, 128], fp32)
            for k in range(2):
                nc.sync.dma_start(w0_sb[k][:], W0[128 * k : 128 * (k + 1), :])
                nc.sync.dma_start(a0_sb[k][:], A0[128 * k : 128 * (k + 1), :])
            nc.sync.dma_start(eye_sb[:], eye[:])

            for t in range(n_t):
                c0 = t * TW
                cw = min(TW, NP - c0)
                xt = [pool.tile([128, TW], fp32, tag=f"xt{k}", name=f"xt{k}") for k in range(2)]
                for k in range(2):
                    nc.sync.dma_start(
                        xt[k][:, :cw], xT[128 * k : 128 * (k + 1), c0 : c0 + cw]
                    )
                hT = [pool.tile([128, TW], fp32, tag=f"ht{m}", name=f"ht{m}") for m in range(2)]
                for m in range(2):
                    ps = pp.tile([128, TW], fp32, tag=f"ps{m}", name=f"ps{m}")
                    for k in range(2):
                        nc.tensor.matmul(
                            ps[:, :cw],
                            w0_sb[k][:, 128 * m : 128 * (m + 1)],
                            xt[k][:, :cw],
                            start=(k == 0),
                            stop=(k == 1),
                        )
                    nc.vector.tensor_copy(hT[m][:, :cw], ps[:, :cw])

                nq = (cw + 127) // 128
                for q in range(nq):
                    q0 = q * 128
                    qw = min(128, cw - q0)
                    # alphas: [qw, 8] = h_rows @ A0
                    pa = pp1.tile([128, 8], fp32, tag="pa")
                    for k in range(2):
                        nc.tensor.matmul(
                            pa[:qw, :],
                            hT[k][:, q0 : q0 + qw],
                            a0_sb[k][:],
                            start=(k == 0),
                            stop=(k == 1),
                        )
                    R = pool.tile([128, ELEM0], fp32, tag="rows")
                    for m in range(2):
                        pt = pp1.tile([128, 128], fp32, tag=f"pt{m}", name=f"pt{m}")
                        nc.tensor.transpose(
                            pt[:qw, :], hT[m][:, q0 : q0 + qw], eye_sb[:]
                        )
                        nc.vector.tensor_copy(
                            R[:qw, 128 * m : 128 * (m + 1)], pt[:qw, :]
                        )
                    nc.vector.tensor_copy(R[:qw, 256:260], pa[:qw, 0:4])
                    nc.vector.memset(R[:qw, 260:ELEM0], 0.0)
                    Dt = pool.tile([128, 64], fp32, tag="dtab")
                    nc.vector.tensor_copy(Dt[:qw, 0:4], pa[:qw, 4:8])
                    nc.vector.memset(Dt[:qw, 4:64], 0.0)
                    r0 = c0 + q0
                    nc.sync.dma_start(table0[r0 : r0 + qw, :], R[:qw, :])
                    nc.sync.dma_start(adtab0[r0 : r0 + qw, :], Dt[:qw, :])
    nc.compile()
    return nc


# ------------------------------------------------------------ edge machinery


def _edge_pass(nc, tc, d, table, adtab, gl, gh, dl, dh, rl, rh, elem, nfeat, nhead, fin):
    """Dst-sorted edge pass: per 128-edge chunk, gather source rows, compute
    softmax weights, weight rows, and segment-reduce into PSUM via a one-hot
    matmul (lhsT = onehot[edge, dst-in-tile]); per dst tile the lo/hi-stream
    chunks accumulate into one PSUM tile, then `fin(t, psum)` consumes it.

    gl/gh: gather idx dram [128, *], dl/dh: adtab idx, rl/rh: dstrel fp32.
    """
    NP, SPLIT, NROWS = d["NLOC_PAD"], d["SPLIT"], d["N_TAB"]
    K_LO, K_HI = d["K_LO"], d["K_HI"]
    NT = NP // 128
    CPC = 8  # chunks per gather call
    RW = nfeat + nhead  # matmul rhs width

    with (
        tc.tile_pool(name="eidx", bufs=1) as ipool,
        tc.tile_pool(name="edge", bufs=3) as pool,
        tc.tile_pool(name="epsum", bufs=2, space="PSUM") as pp,
    ):
        iota_sb = ipool.tile([128, 128], fp32)
        nc.sync.dma_start(iota_sb[:], d["iota_dram"][:])
        streams = []
        for s, (gi_d, di_d, rr_d, K) in enumerate(
            [(gl, dl, rl, K_LO), (gh, dh, rh, K_HI)]
        ):
            nch = NT * K
            gi = ipool.tile([128, nch * 8], i16, name=f"gi{s}")
            di = ipool.tile([128, nch * 8], i16, name=f"di{s}")
            rr = ipool.tile([128, nch], fp32, name=f"rr{s}")
            nc.sync.dma_start(gi[:], gi_d[:])
            nc.sync.dma_start(di[:], di_d[:])
            nc.sync.dma_start(rr[:], rr_d[:])
            base = table[0:SPLIT, :] if s == 0 else table[SPLIT:NROWS, :]
            streams.append(dict(gi=gi, di=di, rr=rr, K=K, base=base, ncalls=0, tiles={}))

        def emit_call(st, call):
            c0 = call * CPC
            nch = min(CPC, NT * st["K"] - c0)
            ne = nch * 128
            G = pool.tile([128, CPC, elem], fp32, tag="G", name="G")
            D = pool.tile([128, CPC, 64], fp32, tag="D", name="D")
            OH = pool.tile([128, CPC, 128], fp32, tag="OH", name="OH")
            ew = pool.tile([128, CPC, nhead], fp32, tag="ew", name="ew")
            nc.gpsimd.dma_gather(
                G[:, :nch, :], st["base"], st["gi"][:, c0 * 8 : c0 * 8 + ne // 16],
                ne, ne, elem,
            )
            nc.gpsimd.dma_gather(
                D[:, :nch, :], adtab[:], st["di"][:, c0 * 8 : c0 * 8 + ne // 16],
                ne, ne, 64,
            )
            nc.vector.tensor_tensor(
                ew[:, :nch, :], G[:, :nch, nfeat : nfeat + nhead],
                D[:, :nch, 0:nhead], op=Alu.add,
            )
            nc.vector.scalar_tensor_tensor(
                ew[:, :nch, :], ew[:, :nch, :], NEG_SLOPE, ew[:, :nch, :],
                op0=Alu.mult, op1=Alu.max,
            )
            nc.scalar.activation(ew[:, :nch, :], ew[:, :nch, :], Act.Exp)
            g4 = G[:, :nch, 0:nfeat].rearrange("p c (h e) -> p c h e", h=nhead)
            wb = (
                ew[:, :nch, :]
                .unsqueeze(3)
                .broadcast_to([128, nch, nhead, nfeat // nhead])
            )
            nc.vector.tensor_tensor(g4, g4, wb, op=Alu.mult)
            nc.vector.tensor_copy(G[:, :nch, nfeat : nfeat + nhead], ew[:, :nch, :])
            rb = (
                st["rr"][:, c0 : c0 + nch]
                .unsqueeze(2)
                .broadcast_to([128, nch, 128])
            )
            ib = iota_sb[:].unsqueeze(1).broadcast_to([128, nch, 128])
            nc.vector.tensor_tensor(OH[:, :nch, :], rb, ib, op=Alu.is_equal)
            return G, OH

        for t in range(NT):
            ps = pp.tile([128, RW], fp32, tag="ps", name="ps")
            first = True
            for st in streams:
                K = st["K"]
                for k in range(K):
                    c = t * K + k
                    call, cin = c // CPC, c % CPC
                    if call >= st["ncalls"]:
                        st["tiles"][call] = emit_call(st, call)
                        st["ncalls"] = call + 1
                        st["tiles"].pop(call - 3, None)
                    G, OH = st["tiles"][call]
                    last = st is streams[1] and k == K - 1
                    nc.tensor.matmul(
                        ps[:],
                        OH[:, cin, :],
                        G[:, cin, 0:RW],
                        start=first,
                        stop=last,
                    )
                    first = False
            fin(t, ps)


# ---------------------------------------------------------------- launch 2


def build_layer0_edges(d):
    """Layer-0 edge pass -> finalize (softmax-div + bias + ELU) fused per dst
    tile -> h1 = h0' @ W1 (+alphas) -> table1 rows + alphaD1 table."""
    nc = bacc.Bacc(None, target_bir_lowering=False, debug=False)
    NP, ELEM0, ELEM1 = d["NLOC_PAD"], d["ELEM0"], d["ELEM1"]
    HID, C_OUT, H, DH = d["HID"], d["C_OUT"], d["H"], d["DH"]
    NT = NP // 128

    table0 = nc.dram_tensor("table0", [d["N_TAB"], ELEM0], fp32, kind="ExternalInput")
    adtab0 = nc.dram_tensor("adtab0", [NP, 64], fp32, kind="ExternalInput")
    gl = nc.dram_tensor("gl", [128, NT * d["K_LO"] * 8], i16, kind="ExternalInput")
    gh = nc.dram_tensor("gh", [128, NT * d["K_HI"] * 8], i16, kind="ExternalInput")
    dl = nc.dram_tensor("dl", [128, NT * d["K_LO"] * 8], i16, kind="ExternalInput")
    dh = nc.dram_tensor("dh", [128, NT * d["K_HI"] * 8], i16, kind="ExternalInput")
    rl = nc.dram_tensor("rl", [128, NT * d["K_LO"]], fp32, kind="ExternalInput")
    rh = nc.dram_tensor("rh", [128, NT * d["K_HI"]], fp32, kind="ExternalInput")
    iota = nc.dram_tensor("iota", [128, 128], fp32, kind="ExternalInput")
    W1 = nc.dram_tensor("W1", [HID, C_OUT], fp32, kind="ExternalInput")
    A1 = nc.dram_tensor("A1", [C_OUT, 2], fp32, kind="ExternalInput")
    b0r = nc.dram_tensor("b0r", [128, HID], fp32, kind="ExternalInput")
    eye = nc.dram_tensor("eye", [128, 128], fp32, kind="ExternalInput")
    table1 = nc.dram_tensor("table1", [NP, ELEM1], fp32, kind="ExternalOutput")
    adtab1 = nc.dram_tensor("adtab1", [NP, 64], fp32, kind="ExternalOutput")
    d = dict(d, iota_dram=iota)

    with tile.TileContext(nc) as tc:
        with (
            tc.tile_pool(name="fconst", bufs=1) as cpool,
            tc.tile_pool(name="fin", bufs=3) as pool,
            tc.tile_pool(name="h0all", bufs=1) as hpool,
            tc.tile_pool(name="fpsum", bufs=1, space="PSUM") as pp,
        ):
            b0_sb = cpool.tile([128, HID], fp32)
            nc.sync.dma_start(b0_sb[:], b0r[:])
            H0 = hpool.tile([128, NT, HID], fp32)

            def fin0(t, ps):
                dn = pool.tile([128, H], fp32, tag="dn", name="dn")
                nc.vector.tensor_scalar_add(dn[:], ps[:, HID : HID + H], EPS)
                rec = pool.tile([128, H], fp32, tag="rec", name="rec")
                nc.vector.reciprocal(rec[:], dn[:])
                f4 = ps[:, 0:HID].rearrange("p (h e) -> p h e", h=H)
                rb = rec[:].unsqueeze(2).broadcast_to([128, H, DH])
                hrow = H0[:, t, :]
                nc.vector.tensor_tensor(
                    hrow.rearrange("p (h e) -> p h e", h=H), f4, rb, op=Alu.mult
                )
                nc.vector.tensor_tensor(hrow, hrow, b0_sb[:], op=Alu.add)
                tn = pool.tile([128, HID], fp32, tag="tn", name="tn")
                nc.vector.tensor_scalar_min(tn[:], hrow, 0.0)
                nc.scalar.activation(tn[:], tn[:], Act.Exp)
                tp = pool.tile([128, HID], fp32, tag="tp", name="tp")
                nc.vector.tensor_scalar_max(tp[:], hrow, 0.0)
                nc.vector.scalar_tensor_tensor(
                    hrow, tn[:], -1.0, tp[:], op0=Alu.add, op1=Alu.add
                )

            _edge_pass(
                nc, tc, d, table0, adtab0, gl, gh, dl, dh, rl, rh, ELEM0, HID, H, fin0
            )

            w1_sb = [
                cpool.tile([128, C_OUT], fp32, tag=f"w1_{k}", name=f"w1_{k}")
                for k in range(2)
            ]
            for k in range(2):
                nc.sync.dma_start(w1_sb[k][:], W1[128 * k : 128 * (k + 1), :])
            a1_sb = cpool.tile([C_OUT, 2], fp32)
            nc.sync.dma_start(a1_sb[:], A1[:])
            eye_sb = cpool.tile([128, 128], fp32)
            nc.sync.dma_start(eye_sb[:], eye[:])

            for r in range(NT):
                h0T = [
                    pool.tile([128, 128], fp32, tag=f"h0T{k}", name=f"h0T{k}")
                    for k in range(2)
                ]
                for k in range(2):
                    pt = pp.tile([128, 128], fp32, tag="pt", name="pt")
                    nc.tensor.transpose(
                        pt[:], H0[:, r, 128 * k : 128 * (k + 1)], eye_sb[:]
                    )
                    nc.vector.tensor_copy(h0T[k][:], pt[:])
                ph1 = pp.tile([C_OUT, 128], fp32, tag="ph1", name="ph1")
                for k in range(2):
                    nc.tensor.matmul(
                        ph1[:], w1_sb[k][:], h0T[k][:], start=(k == 0), stop=(k == 1)
                    )
                h1T = pool.tile([C_OUT, 128], fp32, tag="h1T", name="h1T")
                nc.vector.tensor_copy(h1T[:], ph1[:])
                pal = pp.tile([128, 2], fp32, tag="pal", name="pal")
                nc.tensor.matmul(pal[:], h1T[:], a1_sb[:], start=True, stop=True)
                ptr = pp.tile([128, C_OUT], fp32, tag="ptr", name="ptr")
                nc.tensor.transpose(ptr[:, :], h1T[:, :], eye_sb[:C_OUT, :C_OUT])
                R1 = pool.tile([128, ELEM1], fp32, tag="R1", name="R1")
                nc.vector.tensor_copy(R1[:, 0:C_OUT], ptr[:])
                nc.vector.tensor_copy(R1[:, C_OUT : C_OUT + 1], pal[:, 0:1])
                nc.vector.memset(R1[:, C_OUT + 1 : ELEM1], 0.0)
                D1 = pool.tile([128, 64], fp32, tag="D1", name="D1")
                nc.vector.tensor_copy(D1[:, 0:1], pal[:, 1:2])
                nc.vector.memset(D1[:, 1:64], 0.0)
                nc.sync.dma_start(table1[128 * r : 128 * (r + 1), :], R1[:])
                nc.sync.dma_start(adtab1[128 * r : 128 * (r + 1), :], D1[:])
    nc.compile()
    return nc


# ---------------------------------------------------------------- launch 3


def build_layer1_edges(d):
    """Layer-1 edge pass with fused finalize -> output shard."""
    nc = bacc.Bacc(None, target_bir_lowering=False, debug=False)
    NP, ELEM1, C_OUT = d["NLOC_PAD"], d["ELEM1"], d["C_OUT"]
    NT = NP // 128

    table1 = nc.dram_tensor("table1", [d["N_TAB"], ELEM1], fp32, kind="ExternalInput")
    adtab1 = nc.dram_tensor("adtab1", [NP, 64], fp32, kind="ExternalInput")
    gl = nc.dram_tensor("gl", [128, NT * d["K_LO"] * 8], i16, kind="ExternalInput")
    gh = nc.dram_tensor("gh", [128, NT * d["K_HI"] * 8], i16, kind="ExternalInput")
    dl = nc.dram_tensor("dl", [128, NT * d["K_LO"] * 8], i16, kind="ExternalInput")
    dh = nc.dram_tensor("dh", [128, NT * d["K_HI"] * 8], i16, kind="ExternalInput")
    rl = nc.dram_tensor("rl", [128, NT * d["K_LO"]], fp32, kind="ExternalInput")
    rh = nc.dram_tensor("rh", [128, NT * d["K_HI"]], fp32, kind="ExternalInput")
    iota = nc.dram_tensor("iota", [128, 128], fp32, kind="ExternalInput")
    b1r = nc.dram_tensor("b1r", [128, C_OUT], fp32, kind="ExternalInput")
    out = nc.dram_tensor("out", [NP, C_OUT], fp32, kind="ExternalOutput")
    d = dict(d, iota_dram=iota)

    with tile.TileContext(nc) as tc:
        with (
            tc.tile_pool(name="oconst", bufs=1) as cpool,
            tc.tile_pool(name="ofin", bufs=3) as pool,
        ):
            b1_sb = cpool.tile([128, C_OUT], fp32)
            nc.sync.dma_start(b1_sb[:], b1r[:])

            def fin1(t, ps):
                dn = pool.tile([128, 1], fp32, tag="dn", name="dn")
                nc.vector.tensor_scalar_add(dn[:], ps[:, C_OUT : C_OUT + 1], EPS)
                rec = pool.tile([128, 1], fp32, tag="rec", name="rec")
                nc.vector.reciprocal(rec[:], dn[:])
                O = pool.tile([128, C_OUT], fp32, tag="O", name="O")
                rb = rec[:].broadcast_to([128, C_OUT])
                nc.vector.tensor_tensor(O[:], ps[:, 0:C_OUT], rb, op=Alu.mult)
                nc.vector.tensor_tensor(O[:], O[:], b1_sb[:], op=Alu.add)
                nc.sync.dma_start(out[128 * t : 128 * (t + 1), :], O[:])

            _edge_pass(
                nc, tc, d, table1, adtab1, gl, gh, dl, dh, rl, rh, ELEM1, C_OUT, 1, fin1
            )
    nc.compile()
    return nc


# ------------------------------------------------------------ host plumbing


def _wrap_idx(idx):
    """idx[j] -> [j%16, j//16], replicated across the 8 q7 core groups."""
    a = idx.reshape(-1, 16).T.astype(np.int16)
    return np.tile(a, (8, 1))


def _prep_edges(edge_index, d):
    """Partition edges by dst shard; per core split by src < SPLIT (int16
    gather range), group by 128-row dst tile, sort by dst, and pad each
    (tile, stream) segment to the global max chunk count K_LO / K_HI.
    Returns per-core wrapped index arrays for the edge pass."""
    N, NLOC, NP = d["N"], d["NLOC"], d["NLOC_PAD"]
    SPLIT = d["SPLIT"]
    NT = NP // 128
    src = np.concatenate([edge_index[0], np.arange(N, dtype=np.int64)])
    dst = np.concatenate([edge_index[1], np.arange(N, dtype=np.int64)])
    if d.get("PAD_BLOCKS"):  # merged layout: row(n) = (n//NLOC)*NP + n%NLOC
        srow = (src // NLOC) * NP + (src % NLOC)
    else:
        srow = src
    core = dst // NLOC
    per_core = []
    kmax = [1, 1]
    for c in range(NCORES):
        m = core == c
        s, t = srow[m], dst[m] - c * NLOC
        order = np.argsort(t, kind="stable")
        s, t = s[order], t[order]
        lo = s < SPLIT
        segs = []
        for sm, base in ((lo, 0), (~lo, SPLIT)):
            ss, tt = s[sm] - base, t[sm]
            tilid = tt // 128
            counts = np.bincount(tilid, minlength=NT)
            segs.append((ss, tt, counts))
        per_core.append(segs)
        for si in range(2):
            kmax[si] = max(kmax[si], int(np.ceil(per_core[c][si][2].max() / 128)))
    K_LO, K_HI = kmax
    res = []
    for c in range(NCORES):
        arrs = []
        for si, K in ((0, K_LO), (1, K_HI)):
            ss, tt, counts = per_core[c][si]
            g = np.zeros((NT, K * 128), np.int64)
            dd = np.zeros((NT, K * 128), np.int64)
            rr = np.full((NT, K * 128), -1.0, np.float32)
            offs = np.concatenate([[0], np.cumsum(counts)])
            for tl in range(NT):
                n = counts[tl]
                g[tl, :n] = ss[offs[tl] : offs[tl] + n]
                dd[tl, :n] = tt[offs[tl] : offs[tl] + n]
                rr[tl, :n] = (tt[offs[tl] : offs[tl] + n] - 128 * tl).astype(
                    np.float32
                )
            arrs.append(
                (
                    _wrap_idx(g.ravel()),
                    _wrap_idx(dd.ravel()),
                    np.ascontiguousarray(
                        rr.reshape(NT * K, 128).T
                    ),  # [128, nchunks]
                )
            )
        res.append(arrs)
    return K_LO, K_HI, res


def _build_A0(att_src, att_dst):
    H, DH = att_src.shape
    A = np.zeros((H * DH, 2 * H), np.float32)
    for h in range(H):
        A[h * DH : (h + 1) * DH, h] = att_src[h]
        A[h * DH : (h + 1) * DH, H + h] = att_dst[h]
    return A


_cache = {}
LAST_PROFILE = {}


def _run(nc, in_maps, core_ids, label):
    trace = bool(int(os.environ.get("GAT_PROFILE", "0")))
    if trace:
        try:
            import sys

            import profile_hook

            profile_hook.install()
            import concourse.bass_utils as bu

            bu.upload_artifacts = lambda tmpdir: "local://skipped"
            br = run_bass_kernel_spmd(nc, in_maps, core_ids, trace=True)
            LAST_PROFILE[label] = br.exec_time_ns
            return br.results
        except Exception as e:  # fall back to untraced
            print(f"traced run failed ({e!r}); untraced retry", file=sys.stderr)
    br = run_bass_kernel_spmd(nc, in_maps, core_ids)
    LAST_PROFILE[label] = br.exec_time_ns
    return br.results


def kernel(x, edge_index, W0, att_src0, att_dst0, b0, W1, att_src1, att_dst1, b1):
    x = np.asarray(x, np.float32)
    edge_index = np.asarray(edge_index)
    d = _dims_full()
    d["N_TAB"] = d["N"]
    K_LO, K_HI, idx_arrs = _prep_edges(edge_index, d)
    d["K_LO"], d["K_HI"] = K_LO, K_HI

    key = (K_LO, K_HI)
    if key not in _cache:
        _cache[key] = (
            build_phase_a(d),
            build_layer0_edges(d),
            build_layer1_edges(d),
        )
    nc1, nc2, nc3 = _cache[key]

    N, NLOC, NP = d["N"], d["NLOC"], d["NLOC_PAD"]
    eye = np.eye(128, dtype=np.float32)
    iota = np.tile(np.arange(128, dtype=np.float32)[None, :], (128, 1))
    A0 = _build_A0(np.asarray(att_src0), np.asarray(att_dst0))
    A1 = np.stack(
        [np.asarray(att_src1).ravel(), np.asarray(att_dst1).ravel()], axis=1
    ).astype(np.float32)
    b0r = np.tile(np.asarray(b0, np.float32)[None, :], (128, 1))
    b1r = np.tile(np.asarray(b1, np.float32)[None, :], (128, 1))
    core_ids = list(range(NCORES))

    # launch 1: build table0 shards
    in1 = []
    for c in range(NCORES):
        xs = x[c * NLOC : (c + 1) * NLOC]
        xT = np.zeros((d["F_IN"], NP), np.float32)
        xT[:, :NLOC] = xs.T
        in1.append(dict(xT=xT, W0=np.asarray(W0, np.float32), A0=A0, eye=eye))
    r1 = _run(nc1, in1, core_ids, "l1")
    table0 = np.concatenate([r1[c]["table0"][:NLOC] for c in range(NCORES)], axis=0)

    def edge_inputs(c, extra):
        (gl, dl, rl), (gh, dh, rh) = idx_arrs[c]
        return dict(extra, gl=gl, gh=gh, dl=dl, dh=dh, rl=rl, rh=rh, iota=iota)

    # launch 2: layer-0 edges + finalize + table1 shards
    in2 = [
        edge_inputs(
            c,
            dict(
                table0=table0,
                adtab0=r1[c]["adtab0"],
                W1=np.asarray(W1, np.float32),
                A1=A1,
                b0r=b0r,
                eye=eye,
            ),
        )
        for c in range(NCORES)
    ]
    r2 = _run(nc2, in2, core_ids, "l2")
    table1 = np.concatenate([r2[c]["table1"][:NLOC] for c in range(NCORES)], axis=0)

    # launch 3: layer-1 edges + finalize
    in3 = [
        edge_inputs(c, dict(table1=table1, adtab1=r2[c]["adtab1"], b1r=b1r))
        for c in range(NCORES)
    ]
    r3 = _run(nc3, in3, core_ids, "l3")
    out = np.concatenate([r3[c]["out"][:NLOC] for c in range(NCORES)], axis=0)
    return out


# revision 12
# speedup vs baseline: 1.4782x; 1.4782x over previous
"""Two-layer GAT (PyG-style GATConv x2) on 8 Trainium2 NeuronCores.

Sharding: nodes (and their incident edges, by destination) are sharded
across the 8 cores; small weights are replicated. Per-edge source rows are
fetched with SWDGE dma_gather from a row-major bf16 node table in HBM.
Edges are sorted by destination and grouped per 128-row dst tile; each
128-edge chunk is segment-reduced with a one-hot matmul (lhsT =
onehot[edge, dst-in-tile]) accumulating numerator and softmax denominator
in PSUM — no scatter (dma_scatter_add's CCE RMW races on duplicate
indices, losing updates).

Precision: the node-feature payload is bf16; attention alphas travel as
double-bf16 (hi+lo) pairs and are reconstructed in fp32 on chip, so the
softmax logits keep ~fp32 accuracy. alpha_dst is expanded per edge with an
exact 0/1 matmul (transposed one-hot @ per-tile alpha rows).

Three SPMD launches with host-side concat between them:
  1. table0 build:  h0 = x @ W0, alphas -> row table [N, 320] bf16
  2. layer-0 edges: gather/softmax/onehot-matmul -> finalize (ELU) -> table1
  3. layer-1 edges: same -> finalize -> output

Softmax max-subtraction is skipped: logits are O(5*sigma) so exp() stays
comfortably in fp32 range, and the PyG eps (1e-16) is applied identically.
"""

import os

import numpy as np
from contextlib import ExitStack

import concourse.bacc as bacc
import concourse.mybir as mybir
from concourse import tile
from concourse.bass_utils import run_bass_kernel_spmd

fp32 = mybir.dt.float32
bf16 = mybir.dt.bfloat16
i16 = mybir.dt.int16
Alu = mybir.AluOpType
Act = mybir.ActivationFunctionType

NCORES = 8
NEG_SLOPE = 0.2
EPS = 1e-16


def _dims_full():
    return dict(
        N=50000,  # total nodes
        NLOC=6250,  # nodes per core
        NLOC_PAD=6272,  # padded to mult of 128
        F_IN=256,
        HID=256,
        H=4,
        DH=64,
        C_OUT=64,
        # table0 row (bf16): h(256) | as_hi(4) | as_lo(4) | pad -> 384 (768B)
        ELEM0=384,
        # table1 row (bf16): h1(64) | as_hi | as_lo | pad -> 128 (256B)
        ELEM1=128,
        SPLIT=32768,  # int16 gather-index split point
    )


# ---------------------------------------------------------------- launch 1


def _split_hi_lo(nc, pool, pa_slice, n, tag):
    """fp32 [128, n] -> (hi bf16, lo bf16) tiles with hi+lo ~= value."""
    hi = pool.tile([128, n], bf16, tag=f"{tag}hi", name=f"{tag}hi")
    nc.vector.tensor_copy(hi[:], pa_slice)
    hif = pool.tile([128, n], fp32, tag=f"{tag}hif", name=f"{tag}hif")
    nc.vector.tensor_copy(hif[:], hi[:])
    lo = pool.tile([128, n], bf16, tag=f"{tag}lo", name=f"{tag}lo")
    nc.vector.tensor_tensor(lo[:], pa_slice, hif[:], op=Alu.subtract)
    return hi, lo


def build_phase_a(d):
    """Per core: h0 = x_shard @ W0 (+alphas) -> bf16 table0 rows + alphaD."""
    nc = bacc.Bacc(None, target_bir_lowering=False, debug=False)
    NP, F, HID, ELEM0 = d["NLOC_PAD"], d["F_IN"], d["HID"], d["ELEM0"]
    assert F == 256 and HID == 256

    xT = nc.dram_tensor("xT", [F, NP], fp32, kind="ExternalInput")
    W0 = nc.dram_tensor("W0", [F, HID], fp32, kind="ExternalInput")
    A0 = nc.dram_tensor("A0", [HID, 8], fp32, kind="ExternalInput")
    eye = nc.dram_tensor("eye", [128, 128], fp32, kind="ExternalInput")
    table0 = nc.dram_tensor("table0", [NP, ELEM0], bf16, kind="ExternalOutput")
    adtab0 = nc.dram_tensor("adtab0", [NP, 8], bf16, kind="ExternalOutput")

    TW = 512
    n_t = (NP + TW - 1) // TW

    with tile.TileContext(nc) as tc:
        with (
            tc.tile_pool(name="const", bufs=1) as cpool,
            tc.tile_pool(name="work", bufs=3) as pool,
            tc.tile_pool(name="psum", bufs=1, space="PSUM") as pp,
            tc.tile_pool(name="psum1", bufs=2, space="PSUM") as pp1,
        ):
            w0_sb = [
                cpool.tile([128, HID], fp32, tag=f"w0_{k}", name=f"w0_{k}")
                for k in range(2)
            ]
            a0_sb = [
                cpool.tile([128, 8], fp32, tag=f"a0_{k}", name=f"a0_{k}")
                for k in range(2)
            ]
            eye_sb = cpool.tile([128, 128], fp32)
            for k in range(2):
                nc.sync.dma_start(w0_sb[k][:], W0[128 * k : 128 * (k + 1), :])
                nc.sync.dma_start(a0_sb[k][:], A0[128 * k : 128 * (k + 1), :])
            nc.sync.dma_start(eye_sb[:], eye[:])

            for t in range(n_t):
                c0 = t * TW
                cw = min(TW, NP - c0)
                xt = [
                    pool.tile([128, TW], fp32, tag=f"xt{k}", name=f"xt{k}")
                    for k in range(2)
                ]
                for k in range(2):
                    nc.sync.dma_start(
                        xt[k][:, :cw], xT[128 * k : 128 * (k + 1), c0 : c0 + cw]
                    )
                hT = [
                    pool.tile([128, TW], fp32, tag=f"ht{m}", name=f"ht{m}")
                    for m in range(2)
                ]
                for m in range(2):
                    ps = pp.tile([128, TW], fp32, tag=f"ps{m}", name=f"ps{m}")
                    for k in range(2):
                        nc.tensor.matmul(
                            ps[:, :cw],
                            w0_sb[k][:, 128 * m : 128 * (m + 1)],
                            xt[k][:, :cw],
                            start=(k == 0),
                            stop=(k == 1),
                        )
                    nc.vector.tensor_copy(hT[m][:, :cw], ps[:, :cw])

                nq = (cw + 127) // 128
                for q in range(nq):
                    q0 = q * 128
                    qw = min(128, cw - q0)
                    pa = pp1.tile([128, 8], fp32, tag="pa")
                    for k in range(2):
                        nc.tensor.matmul(
                            pa[:qw, :],
                            hT[k][:, q0 : q0 + qw],
                            a0_sb[k][:],
                            start=(k == 0),
                            stop=(k == 1),
                        )
                    R = pool.tile([128, ELEM0], bf16, tag="rows")
                    for m in range(2):
                        pt = pp1.tile([128, 128], fp32, tag=f"pt{m}", name=f"pt{m}")
                        nc.tensor.transpose(
                            pt[:qw, :], hT[m][:, q0 : q0 + qw], eye_sb[:]
                        )
                        nc.vector.tensor_copy(
                            R[:qw, 128 * m : 128 * (m + 1)], pt[:qw, :]
                        )
                    hi, lo = _split_hi_lo(nc, pool, pa[:qw, 0:4], 4, "as")
                    nc.vector.tensor_copy(R[:qw, 256:260], hi[:qw, :])
                    nc.vector.tensor_copy(R[:qw, 260:264], lo[:qw, :])
                    nc.vector.memset(R[:qw, 264:ELEM0], 0.0)
                    Dt = pool.tile([128, 8], bf16, tag="dtab")
                    dhi, dlo = _split_hi_lo(nc, pool, pa[:qw, 4:8], 4, "ad")
                    nc.vector.tensor_copy(Dt[:qw, 0:4], dhi[:qw, :])
                    nc.vector.tensor_copy(Dt[:qw, 4:8], dlo[:qw, :])
                    r0 = c0 + q0
                    nc.sync.dma_start(table0[r0 : r0 + qw, :], R[:qw, :])
                    nc.sync.dma_start(adtab0[r0 : r0 + qw, :], Dt[:qw, :])
    nc.compile()
    return nc


# ------------------------------------------------------------ edge machinery


def _edge_pass(nc, tc, d, table, adtab, gl, gh, rl, rh, elem, nfeat, nhead, fin):
    """Dst-sorted edge pass. Per 128-edge chunk: gather bf16 source rows,
    reconstruct alphas (double-bf16), softmax-weight the rows, and
    segment-reduce into PSUM with a one-hot matmul; per dst tile the lo/hi
    stream chunks accumulate into one PSUM tile, then fin(t, psum) runs.

    PSUM rhs layout: [weighted msg (nfeat) | w per head (nhead)] so columns
    nfeat:nfeat+nhead accumulate the softmax denominators."""
    NP, SPLIT, NROWS = d["NLOC_PAD"], d["SPLIT"], d["N_TAB"]
    K_LO, K_HI = d["K_LO"], d["K_HI"]
    NT = NP // 128
    CPC = 8  # chunks per gather call
    RW = nfeat + nhead

    with (
        tc.tile_pool(name="eidx", bufs=1) as ipool,
        tc.tile_pool(name="edge", bufs=3) as pool,
        tc.tile_pool(name="epsum", bufs=2, space="PSUM") as pp,
        tc.tile_pool(name="epsum2", bufs=3, space="PSUM") as pp2,
    ):
        iota_sb = ipool.tile([128, 128], bf16)
        nc.sync.dma_start(iota_sb[:], d["iota_dram"][:])
        eyeb_sb = ipool.tile([128, 128], bf16)
        nc.sync.dma_start(eyeb_sb[:], d["eyeb_dram"][:])
        streams = []
        for s, (gi_d, rr_d, K) in enumerate([(gl, rl, K_LO), (gh, rh, K_HI)]):
            nch = NT * K
            gi = ipool.tile([128, nch * 8], i16, name=f"gi{s}")
            rr = ipool.tile([128, nch], bf16, name=f"rr{s}")
            nc.sync.dma_start(gi[:], gi_d[:])
            nc.sync.dma_start(rr[:], rr_d[:])
            base = table[0:SPLIT, :] if s == 0 else table[SPLIT:NROWS, :]
            streams.append(dict(gi=gi, rr=rr, K=K, base=base, ncalls=0, tiles={}))

        def emit_call(st, call):
            c0 = call * CPC
            nch = min(CPC, NT * st["K"] - c0)
            ne = nch * 128
            G = pool.tile([128, CPC, elem], bf16, tag="G", name="G")
            OH = pool.tile([128, CPC, 128], bf16, tag="OH", name="OH")
            OHT = pool.tile([128, CPC, 128], bf16, tag="OHT", name="OHT")
            nc.gpsimd.dma_gather(
                G[:, :nch, :],
                st["base"],
                st["gi"][:, c0 * 8 : c0 * 8 + ne // 16],
                ne,
                ne,
                elem,
            )
            rb = st["rr"][:, c0 : c0 + nch].unsqueeze(2).broadcast_to(
                [128, nch, 128]
            )
            ib = iota_sb[:].unsqueeze(1).broadcast_to([128, nch, 128])
            nc.vector.tensor_tensor(OH[:, :nch, :], rb, ib, op=Alu.is_equal)
            for k in range(nch):
                pst = pp2.tile([128, 128], bf16, tag="pst", name="pst")
                nc.tensor.transpose(pst[:], OH[:, k, :], eyeb_sb[:])
                nc.vector.tensor_copy(OHT[:, k, :], pst[:])
            return G, OH, OHT

        for t in range(NT):
            adt = pool.tile([128, 2 * nhead], bf16, tag="adt", name="adt")
            nc.sync.dma_start(adt[:], adtab[128 * t : 128 * (t + 1), :])
            ps = pp.tile([128, RW], fp32, tag="ps", name="ps")
            first = True
            for st in streams:
                K = st["K"]
                for k in range(K):
                    c = t * K + k
                    call, cin = c // CPC, c % CPC
                    if call >= st["ncalls"]:
                        st["tiles"][call] = emit_call(st, call)
                        st["ncalls"] = call + 1
                        st["tiles"].pop(call - 3, None)
                    G, OH, OHT = st["tiles"][call]
                    # alpha_dst expansion: psE[j,:] = adt[dstrel_j,:] (exact)
                    psE = pp2.tile([128, 2 * nhead], fp32, tag="psE", name="psE")
                    nc.tensor.matmul(
                        psE[:], OHT[:, cin, :], adt[:], start=True, stop=True
                    )
                    ew = pool.tile([128, nhead], fp32, tag="ew", name="ew")
                    # e = (as_hi+as_lo) + (ad_hi+ad_lo); leaky relu; exp
                    nc.vector.tensor_tensor(
                        ew[:],
                        G[:, cin, nfeat : nfeat + nhead],
                        G[:, cin, nfeat + nhead : nfeat + 2 * nhead],
                        op=Alu.add,
                    )
                    nc.vector.tensor_tensor(ew[:], ew[:], psE[:, 0:nhead], op=Alu.add)
                    nc.vector.tensor_tensor(
                        ew[:], ew[:], psE[:, nhead : 2 * nhead], op=Alu.add
                    )
                    nc.vector.scalar_tensor_tensor(
                        ew[:], ew[:], NEG_SLOPE, ew[:], op0=Alu.mult, op1=Alu.max
                    )
                    ewb = pool.tile([128, nhead], bf16, tag="ewb", name="ewb")
                    nc.scalar.activation(ewb[:], ew[:], Act.Exp)
                    gm = G[:, cin, 0:nfeat].rearrange("p (h e) -> p h e", h=nhead)
                    wb = ewb[:].unsqueeze(2).broadcast_to(
                        [128, nhead, nfeat // nhead]
                    )
                    nc.vector.tensor_tensor(gm, gm, wb, op=Alu.mult)
                    # denominator columns: overwrite as_hi slots with w
                    nc.vector.tensor_copy(G[:, cin, nfeat : nfeat + nhead], ewb[:])
                    last = st is streams[1] and k == K - 1
                    nc.tensor.matmul(
                        ps[:],
                        OH[:, cin, :],
                        G[:, cin, 0:RW],
                        start=first,
                        stop=last,
                    )
                    first = False
            fin(t, ps)


# ---------------------------------------------------------------- launch 2


def build_layer0_edges(d):
    """Layer-0 edge pass with fused finalize (softmax-div + bias + ELU),
    then h1 = h0' @ W1 (+alphas) -> bf16 table1 rows + alphaD1."""
    nc = bacc.Bacc(None, target_bir_lowering=False, debug=False)
    NP, ELEM0, ELEM1 = d["NLOC_PAD"], d["ELEM0"], d["ELEM1"]
    HID, C_OUT, H, DH = d["HID"], d["C_OUT"], d["H"], d["DH"]
    NT = NP // 128

    table0 = nc.dram_tensor("table0", [d["N_TAB"], ELEM0], bf16, kind="ExternalInput")
    adtab0 = nc.dram_tensor("adtab0", [NP, 8], bf16, kind="ExternalInput")
    gl = nc.dram_tensor("gl", [128, NT * d["K_LO"] * 8], i16, kind="ExternalInput")
    gh = nc.dram_tensor("gh", [128, NT * d["K_HI"] * 8], i16, kind="ExternalInput")
    rl = nc.dram_tensor("rl", [128, NT * d["K_LO"]], bf16, kind="ExternalInput")
    rh = nc.dram_tensor("rh", [128, NT * d["K_HI"]], bf16, kind="ExternalInput")
    iota = nc.dram_tensor("iota", [128, 128], bf16, kind="ExternalInput")
    eyeb = nc.dram_tensor("eyeb", [128, 128], bf16, kind="ExternalInput")
    W1 = nc.dram_tensor("W1", [HID, C_OUT], fp32, kind="ExternalInput")
    A1 = nc.dram_tensor("A1", [C_OUT, 2], fp32, kind="ExternalInput")
    b0r = nc.dram_tensor("b0r", [128, HID], fp32, kind="ExternalInput")
    eye = nc.dram_tensor("eye", [128, 128], fp32, kind="ExternalInput")
    table1 = nc.dram_tensor("table1", [NP, ELEM1], bf16, kind="ExternalOutput")
    adtab1 = nc.dram_tensor("adtab1", [NP, 2], bf16, kind="ExternalOutput")
    d = dict(d, iota_dram=iota, eyeb_dram=eyeb)

    with tile.TileContext(nc) as tc:
        with (
            tc.tile_pool(name="fconst", bufs=1) as cpool,
            tc.tile_pool(name="fin", bufs=3) as pool,
            tc.tile_pool(name="h0all", bufs=1) as hpool,
        ):
            b0_sb = cpool.tile([128, HID], fp32)
            nc.sync.dma_start(b0_sb[:], b0r[:])
            H0 = hpool.tile([128, NT, HID], fp32)

            def fin0(t, ps):
                dn = pool.tile([128, H], fp32, tag="dn", name="dn")
                nc.vector.tensor_scalar_add(dn[:], ps[:, HID : HID + H], EPS)
                rec = pool.tile([128, H], fp32, tag="rec", name="rec")
                nc.vector.reciprocal(rec[:], dn[:])
                f4 = ps[:, 0:HID].rearrange("p (h e) -> p h e", h=H)
                rb = rec[:].unsqueeze(2).broadcast_to([128, H, DH])
                hrow = H0[:, t, :]
                nc.vector.tensor_tensor(
                    hrow.rearrange("p (h e) -> p h e", h=H), f4, rb, op=Alu.mult
                )
                nc.vector.tensor_tensor(hrow, hrow, b0_sb[:], op=Alu.add)
                tn = pool.tile([128, HID], fp32, tag="tn", name="tn")
                nc.vector.tensor_scalar_min(tn[:], hrow, 0.0)
                nc.scalar.activation(tn[:], tn[:], Act.Exp)
                tp = pool.tile([128, HID], fp32, tag="tp", name="tp")
                nc.vector.tensor_scalar_max(tp[:], hrow, 0.0)
                nc.vector.scalar_tensor_tensor(
                    hrow, tn[:], -1.0, tp[:], op0=Alu.add, op1=Alu.add
                )

            _edge_pass(nc, tc, d, table0, adtab0, gl, gh, rl, rh, ELEM0, HID, H, fin0)

            with (
                tc.tile_pool(name="tb1", bufs=3) as tpool,
                tc.tile_pool(name="tb1psum", bufs=2, space="PSUM") as pp,
            ):
                w1_sb = [
                    cpool.tile([128, C_OUT], fp32, tag=f"w1_{k}", name=f"w1_{k}")
                    for k in range(2)
                ]
                for k in range(2):
                    nc.sync.dma_start(w1_sb[k][:], W1[128 * k : 128 * (k + 1), :])
                a1_sb = cpool.tile([C_OUT, 2], fp32)
                nc.sync.dma_start(a1_sb[:], A1[:])
                eye_sb = cpool.tile([128, 128], fp32)
                nc.sync.dma_start(eye_sb[:], eye[:])

                for r in range(NT):
                    h0T = [
                        tpool.tile([128, 128], fp32, tag=f"h0T{k}", name=f"h0T{k}")
                        for k in range(2)
                    ]
                    for k in range(2):
                        pt = pp.tile([128, 128], fp32, tag="pt", name="pt")
                        nc.tensor.transpose(
                            pt[:], H0[:, r, 128 * k : 128 * (k + 1)], eye_sb[:]
                        )
                        nc.vector.tensor_copy(h0T[k][:], pt[:])
                    ph1 = pp.tile([C_OUT, 128], fp32, tag="ph1", name="ph1")
                    for k in range(2):
                        nc.tensor.matmul(
                            ph1[:],
                            w1_sb[k][:],
                            h0T[k][:],
                            start=(k == 0),
                            stop=(k == 1),
                        )
                    h1T = tpool.tile([C_OUT, 128], fp32, tag="h1T", name="h1T")
                    nc.vector.tensor_copy(h1T[:], ph1[:])
                    pal = pp.tile([128, 2], fp32, tag="pal", name="pal")
                    nc.tensor.matmul(pal[:], h1T[:], a1_sb[:], start=True, stop=True)
                    ptr = pp.tile([128, C_OUT], fp32, tag="ptr", name="ptr")
                    nc.tensor.transpose(ptr[:, :], h1T[:, :], eye_sb[:C_OUT, :C_OUT])
                    R1 = tpool.tile([128, ELEM1], bf16, tag="R1", name="R1")
                    nc.vector.tensor_copy(R1[:, 0:C_OUT], ptr[:])
                    hi, lo = _split_hi_lo(nc, tpool, pal[:, 0:1], 1, "as1")
                    nc.vector.tensor_copy(R1[:, C_OUT : C_OUT + 1], hi[:])
                    nc.vector.tensor_copy(R1[:, C_OUT + 1 : C_OUT + 2], lo[:])
                    nc.vector.memset(R1[:, C_OUT + 2 : ELEM1], 0.0)
                    D1 = tpool.tile([128, 2], bf16, tag="D1", name="D1")
                    dhi, dlo = _split_hi_lo(nc, tpool, pal[:, 1:2], 1, "ad1")
                    nc.vector.tensor_copy(D1[:, 0:1], dhi[:])
                    nc.vector.tensor_copy(D1[:, 1:2], dlo[:])
                    nc.sync.dma_start(table1[128 * r : 128 * (r + 1), :], R1[:])
                    nc.sync.dma_start(adtab1[128 * r : 128 * (r + 1), :], D1[:])
    nc.compile()
    return nc


# ---------------------------------------------------------------- launch 3


def build_layer1_edges(d):
    """Layer-1 edge pass with fused finalize -> output shard."""
    nc = bacc.Bacc(None, target_bir_lowering=False, debug=False)
    NP, ELEM1, C_OUT = d["NLOC_PAD"], d["ELEM1"], d["C_OUT"]
    NT = NP // 128

    table1 = nc.dram_tensor("table1", [d["N_TAB"], ELEM1], bf16, kind="ExternalInput")
    adtab1 = nc.dram_tensor("adtab1", [NP, 2], bf16, kind="ExternalInput")
    gl = nc.dram_tensor("gl", [128, NT * d["K_LO"] * 8], i16, kind="ExternalInput")
    gh = nc.dram_tensor("gh", [128, NT * d["K_HI"] * 8], i16, kind="ExternalInput")
    rl = nc.dram_tensor("rl", [128, NT * d["K_LO"]], bf16, kind="ExternalInput")
    rh = nc.dram_tensor("rh", [128, NT * d["K_HI"]], bf16, kind="ExternalInput")
    iota = nc.dram_tensor("iota", [128, 128], bf16, kind="ExternalInput")
    eyeb = nc.dram_tensor("eyeb", [128, 128], bf16, kind="ExternalInput")
    b1r = nc.dram_tensor("b1r", [128, C_OUT], fp32, kind="ExternalInput")
    out = nc.dram_tensor("out", [NP, C_OUT], fp32, kind="ExternalOutput")
    d = dict(d, iota_dram=iota, eyeb_dram=eyeb)

    with tile.TileContext(nc) as tc:
        with (
            tc.tile_pool(name="oconst", bufs=1) as cpool,
            tc.tile_pool(name="ofin", bufs=3) as pool,
        ):
            b1_sb = cpool.tile([128, C_OUT], fp32)
            nc.sync.dma_start(b1_sb[:], b1r[:])

            def fin1(t, ps):
                dn = pool.tile([128, 1], fp32, tag="dn", name="dn")
                nc.vector.tensor_scalar_add(dn[:], ps[:, C_OUT : C_OUT + 1], EPS)
                rec = pool.tile([128, 1], fp32, tag="rec", name="rec")
                nc.vector.reciprocal(rec[:], dn[:])
                O = pool.tile([128, C_OUT], fp32, tag="O", name="O")
                rb = rec[:].broadcast_to([128, C_OUT])
                nc.vector.tensor_tensor(O[:], ps[:, 0:C_OUT], rb, op=Alu.mult)
                nc.vector.tensor_tensor(O[:], O[:], b1_sb[:], op=Alu.add)
                nc.sync.dma_start(out[128 * t : 128 * (t + 1), :], O[:])

            _edge_pass(nc, tc, d, table1, adtab1, gl, gh, rl, rh, ELEM1, C_OUT, 1, fin1)
    nc.compile()
    return nc


# ------------------------------------------------------------ host plumbing


def _wrap_idx(idx):
    """idx[j] -> [j%16, j//16], replicated across the 8 q7 core groups."""
    a = idx.reshape(-1, 16).T.astype(np.int16)
    return np.tile(a, (8, 1))


def _prep_edges(edge_index, d):
    """Partition edges by dst shard; per core split by src < SPLIT (int16
    gather range), group by 128-row dst tile (sorted by dst), and pad each
    (tile, stream) segment to the global max chunk count K_LO / K_HI."""
    N, NLOC, NP = d["N"], d["NLOC"], d["NLOC_PAD"]
    SPLIT = d["SPLIT"]
    NT = NP // 128
    src = np.concatenate([edge_index[0], np.arange(N, dtype=np.int64)])
    dst = np.concatenate([edge_index[1], np.arange(N, dtype=np.int64)])
    core = dst // NLOC
    per_core = []
    kmax = [1, 1]
    for c in range(NCORES):
        m = core == c
        s, t = src[m], dst[m] - c * NLOC
        order = np.argsort(t, kind="stable")
        s, t = s[order], t[order]
        lo = s < SPLIT
        segs = []
        for sm, base in ((lo, 0), (~lo, SPLIT)):
            ss, tt = s[sm] - base, t[sm]
            counts = np.bincount(tt // 128, minlength=NT)
            segs.append((ss, tt, counts))
        per_core.append(segs)
        for si in range(2):
            kmax[si] = max(kmax[si], int(np.ceil(per_core[c][si][2].max() / 128)))
    K_LO, K_HI = kmax
    res = []
    for c in range(NCORES):
        arrs = []
        for si, K in ((0, K_LO), (1, K_HI)):
            ss, tt, counts = per_core[c][si]
            g = np.zeros((NT, K * 128), np.int64)
            rr = np.full((NT, K * 128), -1.0, np.float32)
            offs = np.concatenate([[0], np.cumsum(counts)])
            for tl in range(NT):
                n = counts[tl]
                g[tl, :n] = ss[offs[tl] : offs[tl] + n]
                rr[tl, :n] = (tt[offs[tl] : offs[tl] + n] - 128 * tl).astype(
                    np.float32
                )
            arrs.append(
                (
                    _wrap_idx(g.ravel()),
                    np.ascontiguousarray(rr.reshape(NT * K, 128).T),
                )
            )
        res.append(arrs)
    return K_LO, K_HI, res


def _build_A0(att_src, att_dst):
    H, DH = att_src.shape
    A = np.zeros((H * DH, 2 * H), np.float32)
    for h in range(H):
        A[h * DH : (h + 1) * DH, h] = att_src[h]
        A[h * DH : (h + 1) * DH, H + h] = att_dst[h]
    return A


def _bf16(a):
    import ml_dtypes

    return a.astype(ml_dtypes.bfloat16)


_cache = {}
LAST_PROFILE = {}


def _run(nc, in_maps, core_ids, label):
    trace = bool(int(os.environ.get("GAT_PROFILE", "0")))
    if trace:
        try:
            import sys

            import profile_hook

            profile_hook.install()
            import concourse.bass_utils as bu

            bu.upload_artifacts = lambda tmpdir: "local://skipped"
            br = run_bass_kernel_spmd(nc, in_maps, core_ids, trace=True)
            LAST_PROFILE[label] = br.exec_time_ns
            return br.results
        except Exception as e:  # fall back to untraced
            print(f"traced run failed ({e!r}); untraced retry", file=sys.stderr)
    br = run_bass_kernel_spmd(nc, in_maps, core_ids)
    LAST_PROFILE[label] = br.exec_time_ns
    return br.results


def kernel(x, edge_index, W0, att_src0, att_dst0, b0, W1, att_src1, att_dst1, b1):
    x = np.asarray(x, np.float32)
    edge_index = np.asarray(edge_index)
    d = _dims_full()
    d["N_TAB"] = d["N"]
    K_LO, K_HI, idx_arrs = _prep_edges(edge_index, d)
    d["K_LO"], d["K_HI"] = K_LO, K_HI

    key = (K_LO, K_HI)
    if key not in _cache:
        _cache[key] = (
            build_phase_a(d),
            build_layer0_edges(d),
            build_layer1_edges(d),
        )
    nc1, nc2, nc3 = _cache[key]

    N, NLOC, NP = d["N"], d["NLOC"], d["NLOC_PAD"]
    eye = np.eye(128, dtype=np.float32)
    eyeb = _bf16(eye)
    iota = _bf16(np.tile(np.arange(128, dtype=np.float32)[None, :], (128, 1)))
    A0 = _build_A0(np.asarray(att_src0), np.asarray(att_dst0))
    A1 = np.stack(
        [np.asarray(att_src1).ravel(), np.asarray(att_dst1).ravel()], axis=1
    ).astype(np.float32)
    b0r = np.tile(np.asarray(b0, np.float32)[None, :], (128, 1))
    b1r = np.tile(np.asarray(b1, np.float32)[None, :], (128, 1))
    core_ids = list(range(NCORES))

    in1 = []
    for c in range(NCORES):
        xs = x[c * NLOC : (c + 1) * NLOC]
        xT = np.zeros((d["F_IN"], NP), np.float32)
        xT[:, :NLOC] = xs.T
        in1.append(dict(xT=xT, W0=np.asarray(W0, np.float32), A0=A0, eye=eye))
    r1 = _run(nc1, in1, core_ids, "l1")
    table0 = np.concatenate([r1[c]["table0"][:NLOC] for c in range(NCORES)], axis=0)

    def edge_inputs(c, extra):
        (gl, rl), (gh, rh) = idx_arrs[c]
        return dict(
            extra, gl=gl, gh=gh, rl=_bf16(rl), rh=_bf16(rh), iota=iota, eyeb=eyeb
        )

    in2 = [
        edge_inputs(
            c,
            dict(
                table0=table0,
                adtab0=r1[c]["adtab0"],
                W1=np.asarray(W1, np.float32),
                A1=A1,
                b0r=b0r,
                eye=eye,
            ),
        )
        for c in range(NCORES)
    ]
    r2 = _run(nc2, in2, core_ids, "l2")
    table1 = np.concatenate([r2[c]["table1"][:NLOC] for c in range(NCORES)], axis=0)

    in3 = [
        edge_inputs(c, dict(table1=table1, adtab1=r2[c]["adtab1"], b1r=b1r))
        for c in range(NCORES)
    ]
    r3 = _run(nc3, in3, core_ids, "l3")
    out = np.concatenate([r3[c]["out"][:NLOC] for c in range(NCORES)], axis=0)
    return out


# revision 15
# speedup vs baseline: 1.8333x; 1.2402x over previous
"""Two-layer GAT (PyG-style GATConv x2) on 8 Trainium2 NeuronCores.

Sharding: nodes (and their incident edges, by destination) are sharded
across the 8 cores; small weights are replicated. Per-edge source rows are
fetched with SWDGE dma_gather from a row-major bf16 node table in HBM.
Edges are sorted by destination and grouped per 128-row dst tile; each
128-edge chunk is segment-reduced with a one-hot matmul (lhsT =
onehot[edge, dst-in-tile]) accumulating numerator and softmax denominator
in PSUM — no scatter (dma_scatter_add's CCE RMW races on duplicate
indices, losing updates).

Precision: the node-feature payload is bf16; attention alphas travel as
double-bf16 (hi+lo) pairs and are reconstructed in fp32 on chip, so the
softmax logits keep ~fp32 accuracy. alpha_dst is expanded per edge with an
exact 0/1 matmul (transposed one-hot @ per-tile alpha rows).

Three SPMD launches with host-side concat between them:
  1. table0 build:  h0 = x @ W0, alphas -> row table [N, 320] bf16
  2. layer-0 edges: gather/softmax/onehot-matmul -> finalize (ELU) -> table1
  3. layer-1 edges: same -> finalize -> output

Softmax max-subtraction is skipped: logits are O(5*sigma) so exp() stays
comfortably in fp32 range, and the PyG eps (1e-16) is applied identically.
"""

import os

import numpy as np
from contextlib import ExitStack

import concourse.bacc as bacc
import concourse.mybir as mybir
from concourse import tile
from concourse.bass_utils import run_bass_kernel_spmd

fp32 = mybir.dt.float32
bf16 = mybir.dt.bfloat16
i16 = mybir.dt.int16
Alu = mybir.AluOpType
Act = mybir.ActivationFunctionType

NCORES = 8
NEG_SLOPE = 0.2
EPS = 1e-16


def _dims_full():
    return dict(
        N=50000,  # total nodes
        NLOC=6250,  # nodes per core
        NLOC_PAD=6272,  # padded to mult of 128
        F_IN=256,
        HID=256,
        H=4,
        DH=64,
        C_OUT=64,
        # table0 row (bf16): h(256) | as_hi(4) | as_lo(4) | pad -> 384 (768B)
        ELEM0=384,
        # table1 row (bf16): h1(64) | as_hi | as_lo | pad -> 128 (256B)
        ELEM1=128,
        SPLIT=32768,  # int16 gather-index split point
    )


# ---------------------------------------------------------------- launch 1


def _split_hi_lo(nc, pool, pa_slice, n, tag):
    """fp32 [128, n] -> (hi bf16, lo bf16) tiles with hi+lo ~= value."""
    hi = pool.tile([128, n], bf16, tag=f"{tag}hi", name=f"{tag}hi")
    nc.vector.tensor_copy(hi[:], pa_slice)
    hif = pool.tile([128, n], fp32, tag=f"{tag}hif", name=f"{tag}hif")
    nc.vector.tensor_copy(hif[:], hi[:])
    lo = pool.tile([128, n], bf16, tag=f"{tag}lo", name=f"{tag}lo")
    nc.vector.tensor_tensor(lo[:], pa_slice, hif[:], op=Alu.subtract)
    return hi, lo


def build_phase_a(d):
    """Per core: h0 = x_shard @ W0 (+alphas) -> bf16 table0 rows + alphaD."""
    nc = bacc.Bacc(None, target_bir_lowering=False, debug=False, num_swdge_queues=4)
    NP, F, HID, ELEM0 = d["NLOC_PAD"], d["F_IN"], d["HID"], d["ELEM0"]
    assert F == 256 and HID == 256

    xT = nc.dram_tensor("xT", [F, NP], fp32, kind="ExternalInput")
    W0 = nc.dram_tensor("W0", [F, HID], fp32, kind="ExternalInput")
    A0 = nc.dram_tensor("A0", [HID, 8], fp32, kind="ExternalInput")
    eye = nc.dram_tensor("eye", [128, 128], fp32, kind="ExternalInput")
    table0 = nc.dram_tensor("table0", [NP, ELEM0], bf16, kind="ExternalOutput")
    adtab0 = nc.dram_tensor("adtab0", [NP, 8], bf16, kind="ExternalOutput")

    TW = 512
    n_t = (NP + TW - 1) // TW

    with tile.TileContext(nc) as tc:
        with (
            tc.tile_pool(name="const", bufs=1) as cpool,
            tc.tile_pool(name="work", bufs=3) as pool,
            tc.tile_pool(name="psum", bufs=1, space="PSUM") as pp,
            tc.tile_pool(name="psum1", bufs=2, space="PSUM") as pp1,
        ):
            w0_sb = [
                cpool.tile([128, HID], fp32, tag=f"w0_{k}", name=f"w0_{k}")
                for k in range(2)
            ]
            a0_sb = [
                cpool.tile([128, 8], fp32, tag=f"a0_{k}", name=f"a0_{k}")
                for k in range(2)
            ]
            eye_sb = cpool.tile([128, 128], fp32)
            for k in range(2):
                nc.sync.dma_start(w0_sb[k][:], W0[128 * k : 128 * (k + 1), :])
                nc.sync.dma_start(a0_sb[k][:], A0[128 * k : 128 * (k + 1), :])
            nc.sync.dma_start(eye_sb[:], eye[:])

            for t in range(n_t):
                c0 = t * TW
                cw = min(TW, NP - c0)
                xt = [
                    pool.tile([128, TW], fp32, tag=f"xt{k}", name=f"xt{k}")
                    for k in range(2)
                ]
                for k in range(2):
                    nc.sync.dma_start(
                        xt[k][:, :cw], xT[128 * k : 128 * (k + 1), c0 : c0 + cw]
                    )
                hT = [
                    pool.tile([128, TW], fp32, tag=f"ht{m}", name=f"ht{m}")
                    for m in range(2)
                ]
                for m in range(2):
                    ps = pp.tile([128, TW], fp32, tag=f"ps{m}", name=f"ps{m}")
                    for k in range(2):
                        nc.tensor.matmul(
                            ps[:, :cw],
                            w0_sb[k][:, 128 * m : 128 * (m + 1)],
                            xt[k][:, :cw],
                            start=(k == 0),
                            stop=(k == 1),
                        )
                    nc.vector.tensor_copy(hT[m][:, :cw], ps[:, :cw])

                nq = (cw + 127) // 128
                for q in range(nq):
                    q0 = q * 128
                    qw = min(128, cw - q0)
                    pa = pp1.tile([128, 8], fp32, tag="pa")
                    for k in range(2):
                        nc.tensor.matmul(
                            pa[:qw, :],
                            hT[k][:, q0 : q0 + qw],
                            a0_sb[k][:],
                            start=(k == 0),
                            stop=(k == 1),
                        )
                    R = pool.tile([128, ELEM0], bf16, tag="rows")
                    for m in range(2):
                        pt = pp1.tile([128, 128], fp32, tag=f"pt{m}", name=f"pt{m}")
                        nc.tensor.transpose(
                            pt[:qw, :], hT[m][:, q0 : q0 + qw], eye_sb[:]
                        )
                        nc.vector.tensor_copy(
                            R[:qw, 128 * m : 128 * (m + 1)], pt[:qw, :]
                        )
                    hi, lo = _split_hi_lo(nc, pool, pa[:qw, 0:4], 4, "as")
                    nc.vector.tensor_copy(R[:qw, 256:260], hi[:qw, :])
                    nc.vector.tensor_copy(R[:qw, 260:264], lo[:qw, :])
                    nc.vector.memset(R[:qw, 264:ELEM0], 0.0)
                    Dt = pool.tile([128, 8], bf16, tag="dtab")
                    dhi, dlo = _split_hi_lo(nc, pool, pa[:qw, 4:8], 4, "ad")
                    nc.vector.tensor_copy(Dt[:qw, 0:4], dhi[:qw, :])
                    nc.vector.tensor_copy(Dt[:qw, 4:8], dlo[:qw, :])
                    r0 = c0 + q0
                    nc.sync.dma_start(table0[r0 : r0 + qw, :], R[:qw, :])
                    nc.sync.dma_start(adtab0[r0 : r0 + qw, :], Dt[:qw, :])
    nc.compile()
    return nc


# ------------------------------------------------------------ edge machinery


def _edge_pass(nc, tc, d, table, gl, gh, rl, rh, al, ah, elem, nfeat, nhead, fin):
    """Dst-sorted edge pass. Per gather call (8 chunks of 128 edges): fetch
    bf16 source rows (SWDGE gather, striped across the 4 SWDGE queues),
    reconstruct logits from double-bf16 alphas (alpha_dst pre-expanded per
    edge on the host between launches), softmax-weight the rows in one
    batched multiply, and build the per-chunk one-hot matrices in one
    batched compare. Per 128-edge chunk a single matmul (lhsT = onehot)
    segment-reduces messages + denominators into the dst tile's PSUM.

    PSUM rhs layout: [weighted msg (nfeat) | w per head (nhead)]."""
    NP, SPLIT, NROWS = d["NLOC_PAD"], d["SPLIT"], d["N_TAB"]
    K_LO, K_HI = d["K_LO"], d["K_HI"]
    NT = NP // 128
    CPC = 8  # chunks per gather call
    RW = nfeat + nhead

    with (
        tc.tile_pool(name="eidx", bufs=1) as ipool,
        tc.tile_pool(name="edge", bufs=3) as pool,
        tc.tile_pool(name="epsum", bufs=2, space="PSUM") as pp,
    ):
        iota_sb = ipool.tile([128, 128], bf16)
        nc.sync.dma_start(iota_sb[:], d["iota_dram"][:])
        streams = []
        for s, (gi_d, rr_d, ad_d, K) in enumerate(
            [(gl, rl, al, K_LO), (gh, rh, ah, K_HI)]
        ):
            nch = NT * K
            gi = ipool.tile([128, nch * 8], i16, name=f"gi{s}")
            rr = ipool.tile([128, nch], bf16, name=f"rr{s}")
            ad = ipool.tile([128, nch, 2 * nhead], bf16, name=f"ad{s}")
            nc.sync.dma_start(gi[:], gi_d[:])
            nc.sync.dma_start(rr[:], rr_d[:])
            nc.sync.dma_start(ad[:], ad_d[:])
            base = table[0:SPLIT, :] if s == 0 else table[SPLIT:NROWS, :]
            streams.append(
                dict(gi=gi, rr=rr, ad=ad, K=K, base=base, ncalls=0, tiles={}, qn=s)
            )

        def emit_call(st, call):
            c0 = call * CPC
            nch = min(CPC, NT * st["K"] - c0)
            ne = nch * 128
            G = pool.tile([128, CPC, elem], bf16, tag="G", name="G")
            OH = pool.tile([128, CPC, 128], bf16, tag="OH", name="OH")
            nc.gpsimd.dma_gather(
                G[:, :nch, :],
                st["base"],
                st["gi"][:, c0 * 8 : c0 * 8 + ne // 16],
                ne,
                ne,
                elem,
                queue_num=(2 * st["qn"] + call % 2),
            )
            rb = st["rr"][:, c0 : c0 + nch].unsqueeze(2).broadcast_to(
                [128, nch, 128]
            )
            ib = iota_sb[:].unsqueeze(1).broadcast_to([128, nch, 128])
            nc.vector.tensor_tensor(OH[:, :nch, :], rb, ib, op=Alu.is_equal)
            ad = st["ad"]
            ew = pool.tile([128, CPC, nhead], fp32, tag="ew", name="ew")
            # e = (as_hi+as_lo) + (ad_hi+ad_lo); leaky relu; exp
            nc.vector.tensor_tensor(
                ew[:, :nch, :],
                G[:, :nch, nfeat : nfeat + nhead],
                G[:, :nch, nfeat + nhead : nfeat + 2 * nhead],
                op=Alu.add,
            )
            nc.vector.tensor_tensor(
                ew[:, :nch, :],
                ew[:, :nch, :],
                ad[:, c0 : c0 + nch, 0:nhead],
                op=Alu.add,
            )
            nc.vector.tensor_tensor(
                ew[:, :nch, :],
                ew[:, :nch, :],
                ad[:, c0 : c0 + nch, nhead : 2 * nhead],
                op=Alu.add,
            )
            nc.vector.scalar_tensor_tensor(
                ew[:, :nch, :],
                ew[:, :nch, :],
                NEG_SLOPE,
                ew[:, :nch, :],
                op0=Alu.mult,
                op1=Alu.max,
            )
            ewb = pool.tile([128, CPC, nhead], bf16, tag="ewb", name="ewb")
            nc.scalar.activation(ewb[:, :nch, :], ew[:, :nch, :], Act.Exp)
            gm = G[:, :nch, 0:nfeat].rearrange("p c (h e) -> p c h e", h=nhead)
            wb = (
                ewb[:, :nch, :]
                .unsqueeze(3)
                .broadcast_to([128, nch, nhead, nfeat // nhead])
            )
            nc.vector.tensor_tensor(gm, gm, wb, op=Alu.mult)
            nc.vector.tensor_copy(
                G[:, :nch, nfeat : nfeat + nhead], ewb[:, :nch, :]
            )
            return G, OH

        for t in range(NT):
            ps = pp.tile([128, RW], fp32, tag="ps", name="ps")
            first = True
            for st in streams:
                K = st["K"]
                for k in range(K):
                    c = t * K + k
                    call, cin = c // CPC, c % CPC
                    if call >= st["ncalls"]:
                        st["tiles"][call] = emit_call(st, call)
                        st["ncalls"] = call + 1
                        st["tiles"].pop(call - 3, None)
                    G, OH = st["tiles"][call]
                    last = st is streams[1] and k == K - 1
                    nc.tensor.matmul(
                        ps[:],
                        OH[:, cin, :],
                        G[:, cin, 0:RW],
                        start=first,
                        stop=last,
                    )
                    first = False
            fin(t, ps)


# ---------------------------------------------------------------- launch 2


def build_layer0_edges(d):
    """Layer-0 edge pass with fused finalize (softmax-div + bias + ELU),
    then h1 = h0' @ W1 (+alphas) -> bf16 table1 rows + alphaD1."""
    nc = bacc.Bacc(None, target_bir_lowering=False, debug=False, num_swdge_queues=4)
    NP, ELEM0, ELEM1 = d["NLOC_PAD"], d["ELEM0"], d["ELEM1"]
    HID, C_OUT, H, DH = d["HID"], d["C_OUT"], d["H"], d["DH"]
    NT = NP // 128

    table0 = nc.dram_tensor("table0", [d["N_TAB"], ELEM0], bf16, kind="ExternalInput")
    gl = nc.dram_tensor("gl", [128, NT * d["K_LO"] * 8], i16, kind="ExternalInput")
    gh = nc.dram_tensor("gh", [128, NT * d["K_HI"] * 8], i16, kind="ExternalInput")
    rl = nc.dram_tensor("rl", [128, NT * d["K_LO"]], bf16, kind="ExternalInput")
    rh = nc.dram_tensor("rh", [128, NT * d["K_HI"]], bf16, kind="ExternalInput")
    al = nc.dram_tensor("al", [128, NT * d["K_LO"], 2 * H], bf16, kind="ExternalInput")
    ah = nc.dram_tensor("ah", [128, NT * d["K_HI"], 2 * H], bf16, kind="ExternalInput")
    iota = nc.dram_tensor("iota", [128, 128], bf16, kind="ExternalInput")
    W1 = nc.dram_tensor("W1", [HID, C_OUT], fp32, kind="ExternalInput")
    A1 = nc.dram_tensor("A1", [C_OUT, 2], fp32, kind="ExternalInput")
    b0r = nc.dram_tensor("b0r", [128, HID], fp32, kind="ExternalInput")
    eye = nc.dram_tensor("eye", [128, 128], fp32, kind="ExternalInput")
    table1 = nc.dram_tensor("table1", [NP, ELEM1], bf16, kind="ExternalOutput")
    adtab1 = nc.dram_tensor("adtab1", [NP, 2], bf16, kind="ExternalOutput")
    d = dict(d, iota_dram=iota)

    with tile.TileContext(nc) as tc:
        with (
            tc.tile_pool(name="fconst", bufs=1) as cpool,
            tc.tile_pool(name="fin", bufs=3) as pool,
            tc.tile_pool(name="h0all", bufs=1) as hpool,
        ):
            b0_sb = cpool.tile([128, HID], fp32)
            nc.sync.dma_start(b0_sb[:], b0r[:])
            H0 = hpool.tile([128, NT, HID], fp32)

            def fin0(t, ps):
                dn = pool.tile([128, H], fp32, tag="dn", name="dn")
                nc.vector.tensor_scalar_add(dn[:], ps[:, HID : HID + H], EPS)
                rec = pool.tile([128, H], fp32, tag="rec", name="rec")
                nc.vector.reciprocal(rec[:], dn[:])
                f4 = ps[:, 0:HID].rearrange("p (h e) -> p h e", h=H)
                rb = rec[:].unsqueeze(2).broadcast_to([128, H, DH])
                hrow = H0[:, t, :]
                nc.vector.tensor_tensor(
                    hrow.rearrange("p (h e) -> p h e", h=H), f4, rb, op=Alu.mult
                )
                nc.vector.tensor_tensor(hrow, hrow, b0_sb[:], op=Alu.add)
                tn = pool.tile([128, HID], fp32, tag="tn", name="tn")
                nc.vector.tensor_scalar_min(tn[:], hrow, 0.0)
                nc.scalar.activation(tn[:], tn[:], Act.Exp)
                tp = pool.tile([128, HID], fp32, tag="tp", name="tp")
                nc.vector.tensor_scalar_max(tp[:], hrow, 0.0)
                nc.vector.scalar_tensor_tensor(
                    hrow, tn[:], -1.0, tp[:], op0=Alu.add, op1=Alu.add
                )

            _edge_pass(nc, tc, d, table0, gl, gh, rl, rh, al, ah, ELEM0, HID, H, fin0)

            with (
                tc.tile_pool(name="tb1", bufs=3) as tpool,
                tc.tile_pool(name="tb1psum", bufs=2, space="PSUM") as pp,
            ):
                w1_sb = [
                    cpool.tile([128, C_OUT], fp32, tag=f"w1_{k}", name=f"w1_{k}")
                    for k in range(2)
                ]
                for k in range(2):
                    nc.sync.dma_start(w1_sb[k][:], W1[128 * k : 128 * (k + 1), :])
                a1_sb = cpool.tile([C_OUT, 2], fp32)
                nc.sync.dma_start(a1_sb[:], A1[:])
                eye_sb = cpool.tile([128, 128], fp32)
                nc.sync.dma_start(eye_sb[:], eye[:])

                for r in range(NT):
                    h0T = [
                        tpool.tile([128, 128], fp32, tag=f"h0T{k}", name=f"h0T{k}")
                        for k in range(2)
                    ]
                    for k in range(2):
                        pt = pp.tile([128, 128], fp32, tag="pt", name="pt")
                        nc.tensor.transpose(
                            pt[:], H0[:, r, 128 * k : 128 * (k + 1)], eye_sb[:]
                        )
                        nc.vector.tensor_copy(h0T[k][:], pt[:])
                    ph1 = pp.tile([C_OUT, 128], fp32, tag="ph1", name="ph1")
                    for k in range(2):
                        nc.tensor.matmul(
                            ph1[:],
                            w1_sb[k][:],
                            h0T[k][:],
                            start=(k == 0),
                            stop=(k == 1),
                        )
                    h1T = tpool.tile([C_OUT, 128], fp32, tag="h1T", name="h1T")
                    nc.vector.tensor_copy(h1T[:], ph1[:])
                    pal = pp.tile([128, 2], fp32, tag="pal", name="pal")
                    nc.tensor.matmul(pal[:], h1T[:], a1_sb[:], start=True, stop=True)
                    ptr = pp.tile([128, C_OUT], fp32, tag="ptr", name="ptr")
                    nc.tensor.transpose(ptr[:, :], h1T[:, :], eye_sb[:C_OUT, :C_OUT])
                    R1 = tpool.tile([128, ELEM1], bf16, tag="R1", name="R1")
                    nc.vector.tensor_copy(R1[:, 0:C_OUT], ptr[:])
                    hi, lo = _split_hi_lo(nc, tpool, pal[:, 0:1], 1, "as1")
                    nc.vector.tensor_copy(R1[:, C_OUT : C_OUT + 1], hi[:])
                    nc.vector.tensor_copy(R1[:, C_OUT + 1 : C_OUT + 2], lo[:])
                    nc.vector.memset(R1[:, C_OUT + 2 : ELEM1], 0.0)
                    D1 = tpool.tile([128, 2], bf16, tag="D1", name="D1")
                    dhi, dlo = _split_hi_lo(nc, tpool, pal[:, 1:2], 1, "ad1")
                    nc.vector.tensor_copy(D1[:, 0:1], dhi[:])
                    nc.vector.tensor_copy(D1[:, 1:2], dlo[:])
                    nc.sync.dma_start(table1[128 * r : 128 * (r + 1), :], R1[:])
                    nc.sync.dma_start(adtab1[128 * r : 128 * (r + 1), :], D1[:])
    nc.compile()
    return nc


# ---------------------------------------------------------------- launch 3


def build_layer1_edges(d):
    """Layer-1 edge pass with fused finalize -> output shard."""
    nc = bacc.Bacc(None, target_bir_lowering=False, debug=False, num_swdge_queues=4)
    NP, ELEM1, C_OUT = d["NLOC_PAD"], d["ELEM1"], d["C_OUT"]
    NT = NP // 128

    table1 = nc.dram_tensor("table1", [d["N_TAB"], ELEM1], bf16, kind="ExternalInput")
    gl = nc.dram_tensor("gl", [128, NT * d["K_LO"] * 8], i16, kind="ExternalInput")
    gh = nc.dram_tensor("gh", [128, NT * d["K_HI"] * 8], i16, kind="ExternalInput")
    rl = nc.dram_tensor("rl", [128, NT * d["K_LO"]], bf16, kind="ExternalInput")
    rh = nc.dram_tensor("rh", [128, NT * d["K_HI"]], bf16, kind="ExternalInput")
    al = nc.dram_tensor("al", [128, NT * d["K_LO"], 2], bf16, kind="ExternalInput")
    ah = nc.dram_tensor("ah", [128, NT * d["K_HI"], 2], bf16, kind="ExternalInput")
    iota = nc.dram_tensor("iota", [128, 128], bf16, kind="ExternalInput")
    b1r = nc.dram_tensor("b1r", [128, C_OUT], fp32, kind="ExternalInput")
    out = nc.dram_tensor("out", [NP, C_OUT], fp32, kind="ExternalOutput")
    d = dict(d, iota_dram=iota)

    with tile.TileContext(nc) as tc:
        with (
            tc.tile_pool(name="oconst", bufs=1) as cpool,
            tc.tile_pool(name="ofin", bufs=3) as pool,
        ):
            b1_sb = cpool.tile([128, C_OUT], fp32)
            nc.sync.dma_start(b1_sb[:], b1r[:])

            def fin1(t, ps):
                dn = pool.tile([128, 1], fp32, tag="dn", name="dn")
                nc.vector.tensor_scalar_add(dn[:], ps[:, C_OUT : C_OUT + 1], EPS)
                rec = pool.tile([128, 1], fp32, tag="rec", name="rec")
                nc.vector.reciprocal(rec[:], dn[:])
                O = pool.tile([128, C_OUT], fp32, tag="O", name="O")
                rb = rec[:].broadcast_to([128, C_OUT])
                nc.vector.tensor_tensor(O[:], ps[:, 0:C_OUT], rb, op=Alu.mult)
                nc.vector.tensor_tensor(O[:], O[:], b1_sb[:], op=Alu.add)
                nc.sync.dma_start(out[128 * t : 128 * (t + 1), :], O[:])

            _edge_pass(nc, tc, d, table1, gl, gh, rl, rh, al, ah, ELEM1, C_OUT, 1, fin1)
    nc.compile()
    return nc


# ------------------------------------------------------------ host plumbing


def _wrap_idx(idx):
    """idx[j] -> [j%16, j//16], replicated across the 8 q7 core groups."""
    a = idx.reshape(-1, 16).T.astype(np.int16)
    return np.tile(a, (8, 1))


def _prep_edges(edge_index, d):
    """Partition edges by dst shard; per core split by src < SPLIT (int16
    gather range), group by 128-row dst tile (sorted by dst), and pad each
    (tile, stream) segment to the global max chunk count K_LO / K_HI."""
    N, NLOC, NP = d["N"], d["NLOC"], d["NLOC_PAD"]
    SPLIT = d["SPLIT"]
    NT = NP // 128
    src = np.concatenate([edge_index[0], np.arange(N, dtype=np.int64)])
    dst = np.concatenate([edge_index[1], np.arange(N, dtype=np.int64)])
    core = dst // NLOC
    per_core = []
    kmax = [1, 1]
    for c in range(NCORES):
        m = core == c
        s, t = src[m], dst[m] - c * NLOC
        order = np.argsort(t, kind="stable")
        s, t = s[order], t[order]
        lo = s < SPLIT
        segs = []
        for sm, base in ((lo, 0), (~lo, SPLIT)):
            ss, tt = s[sm] - base, t[sm]
            counts = np.bincount(tt // 128, minlength=NT)
            segs.append((ss, tt, counts))
        per_core.append(segs)
        for si in range(2):
            kmax[si] = max(kmax[si], int(np.ceil(per_core[c][si][2].max() / 128)))
    K_LO, K_HI = kmax
    res = []
    for c in range(NCORES):
        arrs = []
        for si, K in ((0, K_LO), (1, K_HI)):
            ss, tt, counts = per_core[c][si]
            g = np.zeros((NT, K * 128), np.int64)
            dd = np.zeros((NT, K * 128), np.int64)
            rr = np.full((NT, K * 128), -1.0, np.float32)
            offs = np.concatenate([[0], np.cumsum(counts)])
            for tl in range(NT):
                n = counts[tl]
                g[tl, :n] = ss[offs[tl] : offs[tl] + n]
                dd[tl, :n] = tt[offs[tl] : offs[tl] + n]
                rr[tl, :n] = (tt[offs[tl] : offs[tl] + n] - 128 * tl).astype(
                    np.float32
                )
            arrs.append(
                (
                    _wrap_idx(g.ravel()),
                    np.ascontiguousarray(rr.reshape(NT * K, 128).T),
                    dd.reshape(NT * K, 128),
                )
            )
        res.append(arrs)
    return K_LO, K_HI, res


def _build_A0(att_src, att_dst):
    H, DH = att_src.shape
    A = np.zeros((H * DH, 2 * H), np.float32)
    for h in range(H):
        A[h * DH : (h + 1) * DH, h] = att_src[h]
        A[h * DH : (h + 1) * DH, H + h] = att_dst[h]
    return A


def _bf16(a):
    import ml_dtypes

    return a.astype(ml_dtypes.bfloat16)


_cache = {}
LAST_PROFILE = {}


def _run(nc, in_maps, core_ids, label):
    trace = bool(int(os.environ.get("GAT_PROFILE", "0")))
    if trace:
        try:
            import sys

            import profile_hook

            profile_hook.install()
            import concourse.bass_utils as bu

            bu.upload_artifacts = lambda tmpdir: "local://skipped"
            br = run_bass_kernel_spmd(nc, in_maps, core_ids, trace=True)
            LAST_PROFILE[label] = br.exec_time_ns
            return br.results
        except Exception as e:  # fall back to untraced
            print(f"traced run failed ({e!r}); untraced retry", file=sys.stderr)
    br = run_bass_kernel_spmd(nc, in_maps, core_ids)
    LAST_PROFILE[label] = br.exec_time_ns
    return br.results


def kernel(x, edge_index, W0, att_src0, att_dst0, b0, W1, att_src1, att_dst1, b1):
    x = np.asarray(x, np.float32)
    edge_index = np.asarray(edge_index)
    d = _dims_full()
    d["N_TAB"] = d["N"]
    K_LO, K_HI, idx_arrs = _prep_edges(edge_index, d)
    d["K_LO"], d["K_HI"] = K_LO, K_HI

    key = (K_LO, K_HI)
    if key not in _cache:
        _cache[key] = (
            build_phase_a(d),
            build_layer0_edges(d),
            build_layer1_edges(d),
        )
    nc1, nc2, nc3 = _cache[key]

    N, NLOC, NP = d["N"], d["NLOC"], d["NLOC_PAD"]
    eye = np.eye(128, dtype=np.float32)
    iota = _bf16(np.tile(np.arange(128, dtype=np.float32)[None, :], (128, 1)))
    A0 = _build_A0(np.asarray(att_src0), np.asarray(att_dst0))
    A1 = np.stack(
        [np.asarray(att_src1).ravel(), np.asarray(att_dst1).ravel()], axis=1
    ).astype(np.float32)
    b0r = np.tile(np.asarray(b0, np.float32)[None, :], (128, 1))
    b1r = np.tile(np.asarray(b1, np.float32)[None, :], (128, 1))
    core_ids = list(range(NCORES))

    in1 = []
    for c in range(NCORES):
        xs = x[c * NLOC : (c + 1) * NLOC]
        xT = np.zeros((d["F_IN"], NP), np.float32)
        xT[:, :NLOC] = xs.T
        in1.append(dict(xT=xT, W0=np.asarray(W0, np.float32), A0=A0, eye=eye))
    r1 = _run(nc1, in1, core_ids, "l1")
    table0 = np.concatenate([r1[c]["table0"][:NLOC] for c in range(NCORES)], axis=0)

    def edge_inputs(c, adtab, extra):
        (gl, rl, ddl), (gh, rh, ddh) = idx_arrs[c]
        al = np.ascontiguousarray(adtab[ddl, :].transpose(1, 0, 2))
        ah = np.ascontiguousarray(adtab[ddh, :].transpose(1, 0, 2))
        return dict(
            extra,
            gl=gl,
            gh=gh,
            rl=_bf16(rl),
            rh=_bf16(rh),
            al=al,
            ah=ah,
            iota=iota,
        )

    in2 = [
        edge_inputs(
            c,
            r1[c]["adtab0"],
            dict(
                table0=table0,
                W1=np.asarray(W1, np.float32),
                A1=A1,
                b0r=b0r,
                eye=eye,
            ),
        )
        for c in range(NCORES)
    ]
    r2 = _run(nc2, in2, core_ids, "l2")
    table1 = np.concatenate([r2[c]["table1"][:NLOC] for c in range(NCORES)], axis=0)

    in3 = [
        edge_inputs(c, r2[c]["adtab1"], dict(table1=table1, b1r=b1r))
        for c in range(NCORES)
    ]
    r3 = _run(nc3, in3, core_ids, "l3")
    out = np.concatenate([r3[c]["out"][:NLOC] for c in range(NCORES)], axis=0)
    return out


# revision 16
# speedup vs baseline: 2.4729x; 1.3489x over previous
"""Two-layer GAT (PyG-style GATConv x2) on 8 Trainium2 NeuronCores.

Sharding: nodes (and their incident edges, by destination) are sharded
across the 8 cores; small weights are replicated. Per-edge source rows are
fetched with SWDGE dma_gather from a row-major bf16 node table in HBM.
Edges are sorted by destination and grouped per 128-row dst tile; each
128-edge chunk is segment-reduced with a one-hot matmul (lhsT =
onehot[edge, dst-in-tile]) accumulating numerator and softmax denominator
in PSUM — no scatter (dma_scatter_add's CCE RMW races on duplicate
indices, losing updates).

Precision: the node-feature payload is bf16; attention alphas travel as
double-bf16 (hi+lo) pairs and are reconstructed in fp32 on chip, so the
softmax logits keep ~fp32 accuracy. alpha_dst is expanded per edge with an
exact 0/1 matmul (transposed one-hot @ per-tile alpha rows).

Three SPMD launches with host-side concat between them:
  1. table0 build:  h0 = x @ W0, alphas -> row table [N, 320] bf16
  2. layer-0 edges: gather/softmax/onehot-matmul -> finalize (ELU) -> table1
  3. layer-1 edges: same -> finalize -> output

Softmax max-subtraction is skipped: logits are O(5*sigma) so exp() stays
comfortably in fp32 range, and the PyG eps (1e-16) is applied identically.
"""

import os

import numpy as np
from contextlib import ExitStack

import concourse.bacc as bacc
import concourse.mybir as mybir
from concourse import tile
from concourse.bass_utils import run_bass_kernel_spmd

fp32 = mybir.dt.float32
bf16 = mybir.dt.bfloat16
i16 = mybir.dt.int16
Alu = mybir.AluOpType
Act = mybir.ActivationFunctionType

NCORES = 8
NEG_SLOPE = 0.2
EPS = 1e-16


def _dims_full():
    return dict(
        N=50000,  # total nodes
        NLOC=6250,  # nodes per core
        NLOC_PAD=6272,  # padded to mult of 128
        F_IN=256,
        HID=256,
        H=4,
        DH=64,
        C_OUT=64,
        # table0 row (bf16): h(256) | as_hi(4) | as_lo(4) | pad -> 384 (768B)
        ELEM0=384,
        # table1 row (bf16): h1(64) | as_hi | as_lo | pad -> 128 (256B)
        ELEM1=128,
        SPLIT=32768,  # int16 gather-index split point
    )


# ---------------------------------------------------------------- launch 1


def _split_hi_lo(nc, pool, pa_slice, n, tag):
    """fp32 [128, n] -> (hi bf16, lo bf16) tiles with hi+lo ~= value."""
    hi = pool.tile([128, n], bf16, tag=f"{tag}hi", name=f"{tag}hi")
    nc.vector.tensor_copy(hi[:], pa_slice)
    hif = pool.tile([128, n], fp32, tag=f"{tag}hif", name=f"{tag}hif")
    nc.vector.tensor_copy(hif[:], hi[:])
    lo = pool.tile([128, n], bf16, tag=f"{tag}lo", name=f"{tag}lo")
    nc.vector.tensor_tensor(lo[:], pa_slice, hif[:], op=Alu.subtract)
    return hi, lo


def build_phase_a(d):
    """Per core: h0 = x_shard @ W0 (+alphas) -> bf16 table0 rows + alphaD."""
    nc = bacc.Bacc(None, target_bir_lowering=False, debug=False, num_swdge_queues=4)
    NP, F, HID, ELEM0 = d["NLOC_PAD"], d["F_IN"], d["HID"], d["ELEM0"]
    assert F == 256 and HID == 256

    xT = nc.dram_tensor("xT", [F, NP], fp32, kind="ExternalInput")
    W0 = nc.dram_tensor("W0", [F, HID], fp32, kind="ExternalInput")
    A0 = nc.dram_tensor("A0", [HID, 8], fp32, kind="ExternalInput")
    eye = nc.dram_tensor("eye", [128, 128], fp32, kind="ExternalInput")
    table0 = nc.dram_tensor("table0", [NP, ELEM0], bf16, kind="ExternalOutput")
    adtab0 = nc.dram_tensor("adtab0", [NP, 8], bf16, kind="ExternalOutput")

    TW = 512
    n_t = (NP + TW - 1) // TW

    with tile.TileContext(nc) as tc:
        with (
            tc.tile_pool(name="const", bufs=1) as cpool,
            tc.tile_pool(name="work", bufs=3) as pool,
            tc.tile_pool(name="psum", bufs=1, space="PSUM") as pp,
            tc.tile_pool(name="psum1", bufs=2, space="PSUM") as pp1,
        ):
            w0_sb = [
                cpool.tile([128, HID], fp32, tag=f"w0_{k}", name=f"w0_{k}")
                for k in range(2)
            ]
            a0_sb = [
                cpool.tile([128, 8], fp32, tag=f"a0_{k}", name=f"a0_{k}")
                for k in range(2)
            ]
            eye_sb = cpool.tile([128, 128], fp32)
            for k in range(2):
                nc.sync.dma_start(w0_sb[k][:], W0[128 * k : 128 * (k + 1), :])
                nc.sync.dma_start(a0_sb[k][:], A0[128 * k : 128 * (k + 1), :])
            nc.sync.dma_start(eye_sb[:], eye[:])

            for t in range(n_t):
                c0 = t * TW
                cw = min(TW, NP - c0)
                xt = [
                    pool.tile([128, TW], fp32, tag=f"xt{k}", name=f"xt{k}")
                    for k in range(2)
                ]
                for k in range(2):
                    nc.sync.dma_start(
                        xt[k][:, :cw], xT[128 * k : 128 * (k + 1), c0 : c0 + cw]
                    )
                hT = [
                    pool.tile([128, TW], fp32, tag=f"ht{m}", name=f"ht{m}")
                    for m in range(2)
                ]
                for m in range(2):
                    ps = pp.tile([128, TW], fp32, tag=f"ps{m}", name=f"ps{m}")
                    for k in range(2):
                        nc.tensor.matmul(
                            ps[:, :cw],
                            w0_sb[k][:, 128 * m : 128 * (m + 1)],
                            xt[k][:, :cw],
                            start=(k == 0),
                            stop=(k == 1),
                        )
                    nc.vector.tensor_copy(hT[m][:, :cw], ps[:, :cw])

                nq = (cw + 127) // 128
                for q in range(nq):
                    q0 = q * 128
                    qw = min(128, cw - q0)
                    pa = pp1.tile([128, 8], fp32, tag="pa")
                    for k in range(2):
                        nc.tensor.matmul(
                            pa[:qw, :],
                            hT[k][:, q0 : q0 + qw],
                            a0_sb[k][:],
                            start=(k == 0),
                            stop=(k == 1),
                        )
                    R = pool.tile([128, ELEM0], bf16, tag="rows")
                    for m in range(2):
                        pt = pp1.tile([128, 128], fp32, tag=f"pt{m}", name=f"pt{m}")
                        nc.tensor.transpose(
                            pt[:qw, :], hT[m][:, q0 : q0 + qw], eye_sb[:]
                        )
                        nc.vector.tensor_copy(
                            R[:qw, 128 * m : 128 * (m + 1)], pt[:qw, :]
                        )
                    hi, lo = _split_hi_lo(nc, pool, pa[:qw, 0:4], 4, "as")
                    nc.vector.tensor_copy(R[:qw, 256:260], hi[:qw, :])
                    nc.vector.tensor_copy(R[:qw, 260:264], lo[:qw, :])
                    nc.vector.memset(R[:qw, 264:ELEM0], 0.0)
                    Dt = pool.tile([128, 8], bf16, tag="dtab")
                    dhi, dlo = _split_hi_lo(nc, pool, pa[:qw, 4:8], 4, "ad")
                    nc.vector.tensor_copy(Dt[:qw, 0:4], dhi[:qw, :])
                    nc.vector.tensor_copy(Dt[:qw, 4:8], dlo[:qw, :])
                    r0 = c0 + q0
                    nc.sync.dma_start(table0[r0 : r0 + qw, :], R[:qw, :])
                    nc.sync.dma_start(adtab0[r0 : r0 + qw, :], Dt[:qw, :])
    nc.compile()
    return nc


# ------------------------------------------------------------ edge machinery


def _edge_pass(nc, tc, d, table, gl, gh, rl, rh, al, ah, elem, nfeat, nhead, fin):
    """Dst-sorted edge pass. Per gather call (8 chunks of 128 edges): fetch
    bf16 source rows (SWDGE gather, striped across the 4 SWDGE queues),
    reconstruct logits from double-bf16 alphas (alpha_dst pre-expanded per
    edge on the host between launches), softmax-weight the rows in one
    batched multiply, and build the per-chunk one-hot matrices in one
    batched compare. Per 128-edge chunk a single matmul (lhsT = onehot)
    segment-reduces messages + denominators into the dst tile's PSUM.

    PSUM rhs layout: [weighted msg (nfeat) | w per head (nhead)]."""
    NP, SPLIT, NROWS = d["NLOC_PAD"], d["SPLIT"], d["N_TAB"]
    K_LO, K_HI = d["K_LO"], d["K_HI"]
    NT = NP // 128
    CPC = 8  # chunks per gather call
    RW = nfeat + nhead

    with (
        tc.tile_pool(name="eidx", bufs=1) as ipool,
        tc.tile_pool(name="edge", bufs=3) as pool,
        tc.tile_pool(name="epsum", bufs=4, space="PSUM") as pp,
    ):
        iota_sb = ipool.tile([128, 128], bf16)
        nc.sync.dma_start(iota_sb[:], d["iota_dram"][:])
        streams = []
        for s, (gi_d, rr_d, ad_d, K) in enumerate(
            [(gl, rl, al, K_LO), (gh, rh, ah, K_HI)]
        ):
            nch = NT * K
            gi = ipool.tile([128, nch * 8], i16, name=f"gi{s}")
            rr = ipool.tile([128, nch], bf16, name=f"rr{s}")
            ad = ipool.tile([128, nch, 2 * nhead], bf16, name=f"ad{s}")
            nc.sync.dma_start(gi[:], gi_d[:])
            nc.sync.dma_start(rr[:], rr_d[:])
            nc.sync.dma_start(ad[:], ad_d[:])
            base = table[0:SPLIT, :] if s == 0 else table[SPLIT:NROWS, :]
            streams.append(
                dict(gi=gi, rr=rr, ad=ad, K=K, base=base, ncalls=0, tiles={}, qn=s)
            )

        def emit_call(st, call):
            c0 = call * CPC
            nch = min(CPC, NT * st["K"] - c0)
            ne = nch * 128
            G = pool.tile([128, CPC, elem], bf16, tag="G", name="G", bufs=6)
            OH = pool.tile([128, CPC, 128], bf16, tag="OH", name="OH", bufs=6)
            nc.gpsimd.dma_gather(
                G[:, :nch, :],
                st["base"],
                st["gi"][:, c0 * 8 : c0 * 8 + ne // 16],
                ne,
                ne,
                elem,
                queue_num=(2 * st["qn"] + call % 2),
            )
            rb = st["rr"][:, c0 : c0 + nch].unsqueeze(2).broadcast_to(
                [128, nch, 128]
            )
            ib = iota_sb[:].unsqueeze(1).broadcast_to([128, nch, 128])
            nc.vector.tensor_tensor(OH[:, :nch, :], rb, ib, op=Alu.is_equal)
            ad = st["ad"]
            ew = pool.tile([128, CPC, nhead], fp32, tag="ew", name="ew", bufs=6)
            # e = (as_hi+as_lo) + (ad_hi+ad_lo); leaky relu; exp
            nc.vector.tensor_tensor(
                ew[:, :nch, :],
                G[:, :nch, nfeat : nfeat + nhead],
                G[:, :nch, nfeat + nhead : nfeat + 2 * nhead],
                op=Alu.add,
            )
            nc.vector.tensor_tensor(
                ew[:, :nch, :],
                ew[:, :nch, :],
                ad[:, c0 : c0 + nch, 0:nhead],
                op=Alu.add,
            )
            nc.vector.tensor_tensor(
                ew[:, :nch, :],
                ew[:, :nch, :],
                ad[:, c0 : c0 + nch, nhead : 2 * nhead],
                op=Alu.add,
            )
            nc.vector.scalar_tensor_tensor(
                ew[:, :nch, :],
                ew[:, :nch, :],
                NEG_SLOPE,
                ew[:, :nch, :],
                op0=Alu.mult,
                op1=Alu.max,
            )
            ewb = pool.tile([128, CPC, nhead], bf16, tag="ewb", name="ewb", bufs=6)
            nc.scalar.activation(ewb[:, :nch, :], ew[:, :nch, :], Act.Exp)
            gm = G[:, :nch, 0:nfeat].rearrange("p c (h e) -> p c h e", h=nhead)
            wb = (
                ewb[:, :nch, :]
                .unsqueeze(3)
                .broadcast_to([128, nch, nhead, nfeat // nhead])
            )
            nc.vector.tensor_tensor(gm, gm, wb, op=Alu.mult)
            nc.vector.tensor_copy(
                G[:, :nch, nfeat : nfeat + nhead], ewb[:, :nch, :]
            )
            return G, OH

        for t in range(NT):
            ps = pp.tile([128, RW], fp32, tag="ps", name="ps")
            first = True
            for st in streams:
                K = st["K"]
                for k in range(K):
                    c = t * K + k
                    call, cin = c // CPC, c % CPC
                    if call >= st["ncalls"]:
                        st["tiles"][call] = emit_call(st, call)
                        st["ncalls"] = call + 1
                        st["tiles"].pop(call - 3, None)
                    G, OH = st["tiles"][call]
                    last = st is streams[1] and k == K - 1
                    nc.tensor.matmul(
                        ps[:],
                        OH[:, cin, :],
                        G[:, cin, 0:RW],
                        start=first,
                        stop=last,
                    )
                    first = False
            fin(t, ps)


# ---------------------------------------------------------------- launch 2


def build_layer0_edges(d):
    """Layer-0 edge pass with fused finalize (softmax-div + bias + ELU),
    then h1 = h0' @ W1 (+alphas) -> bf16 table1 rows + alphaD1."""
    nc = bacc.Bacc(None, target_bir_lowering=False, debug=False, num_swdge_queues=4)
    NP, ELEM0, ELEM1 = d["NLOC_PAD"], d["ELEM0"], d["ELEM1"]
    HID, C_OUT, H, DH = d["HID"], d["C_OUT"], d["H"], d["DH"]
    NT = NP // 128

    table0 = nc.dram_tensor("table0", [d["N_TAB"], ELEM0], bf16, kind="ExternalInput")
    gl = nc.dram_tensor("gl", [128, NT * d["K_LO"] * 8], i16, kind="ExternalInput")
    gh = nc.dram_tensor("gh", [128, NT * d["K_HI"] * 8], i16, kind="ExternalInput")
    rl = nc.dram_tensor("rl", [128, NT * d["K_LO"]], bf16, kind="ExternalInput")
    rh = nc.dram_tensor("rh", [128, NT * d["K_HI"]], bf16, kind="ExternalInput")
    al = nc.dram_tensor("al", [128, NT * d["K_LO"], 2 * H], bf16, kind="ExternalInput")
    ah = nc.dram_tensor("ah", [128, NT * d["K_HI"], 2 * H], bf16, kind="ExternalInput")
    iota = nc.dram_tensor("iota", [128, 128], bf16, kind="ExternalInput")
    W1 = nc.dram_tensor("W1", [HID, C_OUT], fp32, kind="ExternalInput")
    A1 = nc.dram_tensor("A1", [C_OUT, 2], fp32, kind="ExternalInput")
    b0r = nc.dram_tensor("b0r", [128, HID], fp32, kind="ExternalInput")
    eye = nc.dram_tensor("eye", [128, 128], fp32, kind="ExternalInput")
    table1 = nc.dram_tensor("table1", [NP, ELEM1], bf16, kind="ExternalOutput")
    adtab1 = nc.dram_tensor("adtab1", [NP, 2], bf16, kind="ExternalOutput")
    d = dict(d, iota_dram=iota)

    with tile.TileContext(nc) as tc:
        with (
            tc.tile_pool(name="fconst", bufs=1) as cpool,
            tc.tile_pool(name="fin", bufs=3) as pool,
            tc.tile_pool(name="h0all", bufs=1) as hpool,
        ):
            b0_sb = cpool.tile([128, HID], fp32)
            nc.sync.dma_start(b0_sb[:], b0r[:])
            H0 = hpool.tile([128, NT, HID], fp32)

            def fin0(t, ps):
                dn = pool.tile([128, H], fp32, tag="dn", name="dn")
                nc.vector.tensor_scalar_add(dn[:], ps[:, HID : HID + H], EPS)
                rec = pool.tile([128, H], fp32, tag="rec", name="rec")
                nc.vector.reciprocal(rec[:], dn[:])
                f4 = ps[:, 0:HID].rearrange("p (h e) -> p h e", h=H)
                rb = rec[:].unsqueeze(2).broadcast_to([128, H, DH])
                hrow = H0[:, t, :]
                nc.vector.tensor_tensor(
                    hrow.rearrange("p (h e) -> p h e", h=H), f4, rb, op=Alu.mult
                )
                nc.vector.tensor_tensor(hrow, hrow, b0_sb[:], op=Alu.add)
                tn = pool.tile([128, HID], fp32, tag="tn", name="tn")
                nc.vector.tensor_scalar_min(tn[:], hrow, 0.0)
                nc.scalar.activation(tn[:], tn[:], Act.Exp)
                tp = pool.tile([128, HID], fp32, tag="tp", name="tp")
                nc.vector.tensor_scalar_max(tp[:], hrow, 0.0)
                nc.vector.scalar_tensor_tensor(
                    hrow, tn[:], -1.0, tp[:], op0=Alu.add, op1=Alu.add
                )

            _edge_pass(nc, tc, d, table0, gl, gh, rl, rh, al, ah, ELEM0, HID, H, fin0)

            with (
                tc.tile_pool(name="tb1", bufs=3) as tpool,
                tc.tile_pool(name="tb1psum", bufs=2, space="PSUM") as pp,
            ):
                w1_sb = [
                    cpool.tile([128, C_OUT], fp32, tag=f"w1_{k}", name=f"w1_{k}")
                    for k in range(2)
                ]
                for k in range(2):
                    nc.sync.dma_start(w1_sb[k][:], W1[128 * k : 128 * (k + 1), :])
                a1_sb = cpool.tile([C_OUT, 2], fp32)
                nc.sync.dma_start(a1_sb[:], A1[:])
                eye_sb = cpool.tile([128, 128], fp32)
                nc.sync.dma_start(eye_sb[:], eye[:])

                for r in range(NT):
                    h0T = [
                        tpool.tile([128, 128], fp32, tag=f"h0T{k}", name=f"h0T{k}")
                        for k in range(2)
                    ]
                    for k in range(2):
                        pt = pp.tile([128, 128], fp32, tag="pt", name="pt")
                        nc.tensor.transpose(
                            pt[:], H0[:, r, 128 * k : 128 * (k + 1)], eye_sb[:]
                        )
                        nc.vector.tensor_copy(h0T[k][:], pt[:])
                    ph1 = pp.tile([C_OUT, 128], fp32, tag="ph1", name="ph1")
                    for k in range(2):
                        nc.tensor.matmul(
                            ph1[:],
                            w1_sb[k][:],
                            h0T[k][:],
                            start=(k == 0),
                            stop=(k == 1),
                        )
                    h1T = tpool.tile([C_OUT, 128], fp32, tag="h1T", name="h1T")
                    nc.vector.tensor_copy(h1T[:], ph1[:])
                    pal = pp.tile([128, 2], fp32, tag="pal", name="pal")
                    nc.tensor.matmul(pal[:], h1T[:], a1_sb[:], start=True, stop=True)
                    ptr = pp.tile([128, C_OUT], fp32, tag="ptr", name="ptr")
                    nc.tensor.transpose(ptr[:, :], h1T[:, :], eye_sb[:C_OUT, :C_OUT])
                    R1 = tpool.tile([128, ELEM1], bf16, tag="R1", name="R1")
                    nc.vector.tensor_copy(R1[:, 0:C_OUT], ptr[:])
                    hi, lo = _split_hi_lo(nc, tpool, pal[:, 0:1], 1, "as1")
                    nc.vector.tensor_copy(R1[:, C_OUT : C_OUT + 1], hi[:])
                    nc.vector.tensor_copy(R1[:, C_OUT + 1 : C_OUT + 2], lo[:])
                    nc.vector.memset(R1[:, C_OUT + 2 : ELEM1], 0.0)
                    D1 = tpool.tile([128, 2], bf16, tag="D1", name="D1")
                    dhi, dlo = _split_hi_lo(nc, tpool, pal[:, 1:2], 1, "ad1")
                    nc.vector.tensor_copy(D1[:, 0:1], dhi[:])
                    nc.vector.tensor_copy(D1[:, 1:2], dlo[:])
                    nc.sync.dma_start(table1[128 * r : 128 * (r + 1), :], R1[:])
                    nc.sync.dma_start(adtab1[128 * r : 128 * (r + 1), :], D1[:])
    nc.compile()
    return nc


# ---------------------------------------------------------------- launch 3


def build_layer1_edges(d):
    """Layer-1 edge pass with fused finalize -> output shard."""
    nc = bacc.Bacc(None, target_bir_lowering=False, debug=False, num_swdge_queues=4)
    NP, ELEM1, C_OUT = d["NLOC_PAD"], d["ELEM1"], d["C_OUT"]
    NT = NP // 128

    table1 = nc.dram_tensor("table1", [d["N_TAB"], ELEM1], bf16, kind="ExternalInput")
    gl = nc.dram_tensor("gl", [128, NT * d["K_LO"] * 8], i16, kind="ExternalInput")
    gh = nc.dram_tensor("gh", [128, NT * d["K_HI"] * 8], i16, kind="ExternalInput")
    rl = nc.dram_tensor("rl", [128, NT * d["K_LO"]], bf16, kind="ExternalInput")
    rh = nc.dram_tensor("rh", [128, NT * d["K_HI"]], bf16, kind="ExternalInput")
    al = nc.dram_tensor("al", [128, NT * d["K_LO"], 2], bf16, kind="ExternalInput")
    ah = nc.dram_tensor("ah", [128, NT * d["K_HI"], 2], bf16, kind="ExternalInput")
    iota = nc.dram_tensor("iota", [128, 128], bf16, kind="ExternalInput")
    b1r = nc.dram_tensor("b1r", [128, C_OUT], fp32, kind="ExternalInput")
    out = nc.dram_tensor("out", [NP, C_OUT], fp32, kind="ExternalOutput")
    d = dict(d, iota_dram=iota)

    with tile.TileContext(nc) as tc:
        with (
            tc.tile_pool(name="oconst", bufs=1) as cpool,
            tc.tile_pool(name="ofin", bufs=3) as pool,
        ):
            b1_sb = cpool.tile([128, C_OUT], fp32)
            nc.sync.dma_start(b1_sb[:], b1r[:])

            def fin1(t, ps):
                dn = pool.tile([128, 1], fp32, tag="dn", name="dn")
                nc.vector.tensor_scalar_add(dn[:], ps[:, C_OUT : C_OUT + 1], EPS)
                rec = pool.tile([128, 1], fp32, tag="rec", name="rec")
                nc.vector.reciprocal(rec[:], dn[:])
                O = pool.tile([128, C_OUT], fp32, tag="O", name="O")
                rb = rec[:].broadcast_to([128, C_OUT])
                nc.vector.tensor_tensor(O[:], ps[:, 0:C_OUT], rb, op=Alu.mult)
                nc.vector.tensor_tensor(O[:], O[:], b1_sb[:], op=Alu.add)
                nc.sync.dma_start(out[128 * t : 128 * (t + 1), :], O[:])

            _edge_pass(nc, tc, d, table1, gl, gh, rl, rh, al, ah, ELEM1, C_OUT, 1, fin1)
    nc.compile()
    return nc


# ------------------------------------------------------------ host plumbing


def _wrap_idx(idx):
    """idx[j] -> [j%16, j//16], replicated across the 8 q7 core groups."""
    a = idx.reshape(-1, 16).T.astype(np.int16)
    return np.tile(a, (8, 1))


def _prep_edges(edge_index, d):
    """Partition edges by dst shard; per core split by src < SPLIT (int16
    gather range), group by 128-row dst tile (sorted by dst), and pad each
    (tile, stream) segment to the global max chunk count K_LO / K_HI."""
    N, NLOC, NP = d["N"], d["NLOC"], d["NLOC_PAD"]
    SPLIT = d["SPLIT"]
    NT = NP // 128
    src = np.concatenate([edge_index[0], np.arange(N, dtype=np.int64)])
    dst = np.concatenate([edge_index[1], np.arange(N, dtype=np.int64)])
    core = dst // NLOC
    per_core = []
    kmax = [1, 1]
    for c in range(NCORES):
        m = core == c
        s, t = src[m], dst[m] - c * NLOC
        order = np.argsort(t, kind="stable")
        s, t = s[order], t[order]
        lo = s < SPLIT
        segs = []
        for sm, base in ((lo, 0), (~lo, SPLIT)):
            ss, tt = s[sm] - base, t[sm]
            counts = np.bincount(tt // 128, minlength=NT)
            segs.append((ss, tt, counts))
        per_core.append(segs)
        for si in range(2):
            kmax[si] = max(kmax[si], int(np.ceil(per_core[c][si][2].max() / 128)))
    K_LO, K_HI = kmax
    res = []
    for c in range(NCORES):
        arrs = []
        for si, K in ((0, K_LO), (1, K_HI)):
            ss, tt, counts = per_core[c][si]
            g = np.zeros((NT, K * 128), np.int64)
            dd = np.zeros((NT, K * 128), np.int64)
            rr = np.full((NT, K * 128), -1.0, np.float32)
            offs = np.concatenate([[0], np.cumsum(counts)])
            for tl in range(NT):
                n = counts[tl]
                g[tl, :n] = ss[offs[tl] : offs[tl] + n]
                dd[tl, :n] = tt[offs[tl] : offs[tl] + n]
                rr[tl, :n] = (tt[offs[tl] : offs[tl] + n] - 128 * tl).astype(
                    np.float32
                )
            arrs.append(
                (
                    _wrap_idx(g.ravel()),
                    np.ascontiguousarray(rr.reshape(NT * K, 128).T),
                    dd.reshape(NT * K, 128),
                )
            )
        res.append(arrs)
    return K_LO, K_HI, res


def _build_A0(att_src, att_dst):
    H, DH = att_src.shape
    A = np.zeros((H * DH, 2 * H), np.float32)
    for h in range(H):
        A[h * DH : (h + 1) * DH, h] = att_src[h]
        A[h * DH : (h + 1) * DH, H + h] = att_dst[h]
    return A


def _bf16(a):
    import ml_dtypes

    return a.astype(ml_dtypes.bfloat16)


_cache = {}
LAST_PROFILE = {}


def _run(nc, in_maps, core_ids, label):
    trace = bool(int(os.environ.get("GAT_PROFILE", "0")))
    if trace:
        try:
            import sys

            import profile_hook

            profile_hook.install()
            import concourse.bass_utils as bu

            bu.upload_artifacts = lambda tmpdir: "local://skipped"
            br = run_bass_kernel_spmd(nc, in_maps, core_ids, trace=True)
            LAST_PROFILE[label] = br.exec_time_ns
            return br.results
        except Exception as e:  # fall back to untraced
            print(f"traced run failed ({e!r}); untraced retry", file=sys.stderr)
    br = run_bass_kernel_spmd(nc, in_maps, core_ids)
    LAST_PROFILE[label] = br.exec_time_ns
    return br.results


def kernel(x, edge_index, W0, att_src0, att_dst0, b0, W1, att_src1, att_dst1, b1):
    x = np.asarray(x, np.float32)
    edge_index = np.asarray(edge_index)
    d = _dims_full()
    d["N_TAB"] = d["N"]
    K_LO, K_HI, idx_arrs = _prep_edges(edge_index, d)
    d["K_LO"], d["K_HI"] = K_LO, K_HI

    key = (K_LO, K_HI)
    if key not in _cache:
        _cache[key] = (
            build_phase_a(d),
            build_layer0_edges(d),
            build_layer1_edges(d),
        )
    nc1, nc2, nc3 = _cache[key]

    N, NLOC, NP = d["N"], d["NLOC"], d["NLOC_PAD"]
    eye = np.eye(128, dtype=np.float32)
    iota = _bf16(np.tile(np.arange(128, dtype=np.float32)[None, :], (128, 1)))
    A0 = _build_A0(np.asarray(att_src0), np.asarray(att_dst0))
    A1 = np.stack(
        [np.asarray(att_src1).ravel(), np.asarray(att_dst1).ravel()], axis=1
    ).astype(np.float32)
    b0r = np.tile(np.asarray(b0, np.float32)[None, :], (128, 1))
    b1r = np.tile(np.asarray(b1, np.float32)[None, :], (128, 1))
    core_ids = list(range(NCORES))

    in1 = []
    for c in range(NCORES):
        xs = x[c * NLOC : (c + 1) * NLOC]
        xT = np.zeros((d["F_IN"], NP), np.float32)
        xT[:, :NLOC] = xs.T
        in1.append(dict(xT=xT, W0=np.asarray(W0, np.float32), A0=A0, eye=eye))
    r1 = _run(nc1, in1, core_ids, "l1")
    table0 = np.concatenate([r1[c]["table0"][:NLOC] for c in range(NCORES)], axis=0)

    def edge_inputs(c, adtab, extra):
        (gl, rl, ddl), (gh, rh, ddh) = idx_arrs[c]
        al = np.ascontiguousarray(adtab[ddl, :].transpose(1, 0, 2))
        ah = np.ascontiguousarray(adtab[ddh, :].transpose(1, 0, 2))
        return dict(
            extra,
            gl=gl,
            gh=gh,
            rl=_bf16(rl),
            rh=_bf16(rh),
            al=al,
            ah=ah,
            iota=iota,
        )

    in2 = [
        edge_inputs(
            c,
            r1[c]["adtab0"],
            dict(
                table0=table0,
                W1=np.asarray(W1, np.float32),
                A1=A1,
                b0r=b0r,
                eye=eye,
            ),
        )
        for c in range(NCORES)
    ]
    r2 = _run(nc2, in2, core_ids, "l2")
    table1 = np.concatenate([r2[c]["table1"][:NLOC] for c in range(NCORES)], axis=0)

    in3 = [
        edge_inputs(c, r2[c]["adtab1"], dict(table1=table1, b1r=b1r))
        for c in range(NCORES)
    ]
    r3 = _run(nc3, in3, core_ids, "l3")
    out = np.concatenate([r3[c]["out"][:NLOC] for c in range(NCORES)], axis=0)
    return out
